# revision 54
# baseline (speedup 1.0000x reference)
"""Trainium2 Bass kernel for causal multi-head attention with interleaved RoPE.

Problem: B=2, S=2048, D=1024, 16 heads x 64 dims, causal, rope theta=1e4.

Sharding (8 cores): 2-way batch x 4-way head tensor-parallel.
  core i: batch b = i // 4, head group g = i % 4 (heads 4g..4g+3, dims 256).
  Each core computes q/k/v for its heads from x[b], runs causal flash
  attention, and produces a partial output projection outT [D, S].  Host
  sums the 4 partials per batch and transposes.

Performance design (v2):
  - Inputs in bf16 (halves HBM traffic); x streamed in 512-column chunks so
    projections start ~5us in instead of waiting 46us for the full load.
  - Scores: the two heads of a 128-partition group are computed as a
    row-tiled matmul pair (tile_position (0,0)/(64,0)) so both K=64
    contractions run concurrently in the PE array.
  - Both heads' score chunks live in one [128, 2, 512] PSUM tile (2 banks)
    and are EXPed by a single scalar activation -> fewer scalar instructions
    (scalar engine is the bottleneck of the attention phase; concurrent
    scalar activity also throttles the PE to ~1.2GHz, so PE work per chunk
    is halved via pairing).
  - AV: col-tiled pair (tile_position (0,0)/(0,64)) into two separate PSUM
    banks (separate banks because a matmul with start=True clears the
    has_written bits of its whole bank).
  - Softmax denominators: probs are accumulated on the Vector engine into
    sacc, reduced with M=1 ones-matmuls, inverted with one
    reciprocal_approx_fast, and broadcast back with a selection matmul --
    no DRAM round trip.
  - Output projection per q-tile right after normalize, overlapping DMA out.
"""

import os
import sys

sys.path.insert(0, "/opt/trn_rl_repo")

import numpy as np

B = 2
S = 2048
D = 1024
NH = 16
HD = 64
THETA = 10000.0
NCORES = 8
HPC = 4  # heads per core
DC = HPC * HD  # 256 dims per core
GQ = 2  # 128-partition head groups per core
QT = 512  # query tile (free dim)
NQT = S // QT
KC = 128  # key chunk (partition dim)
MASKVAL = -60.0

_CACHE = {}


def _install_axon_ntff_hook():
    """Register antenv.axon_hooks so trace=True (BASS_TRACE=1) works."""
    import types

    if "antenv.axon_hooks" in sys.modules:
        return
    m = types.ModuleType("antenv.axon_hooks")
    _hook = [None]
    m.set_axon_ntff_profile_hook = lambda h: _hook.__setitem__(0, h)
    m.get_axon_ntff_profile_hook = lambda: _hook[0]
    sys.modules["antenv.axon_hooks"] = m
    try:
        import antenv

        antenv.axon_hooks = m
        from trn_agent_boot.trn_boot import _ntff_profile_via_ctypes

        hook = _ntff_profile_via_ctypes("/opt/axon/libaxon_pjrt.so")
        if hook is not None:
            m.set_axon_ntff_profile_hook(hook)
    except Exception:
        pass


def _rope_perm_local():
    """Permutation of one head's 64 dims: original interleaved pair (2i, 2i+1)
    -> t0 at quadrant*32 + (i%16), t1 at quadrant*32 + 16 + (i%16), with
    quadrant = i // 16.  Returns perm such that new[j] = old[perm[j]]."""
    perm = np.zeros(HD, dtype=np.int64)
    for i in range(HD // 2):
        qd, r = divmod(i, 16)
        perm[qd * 32 + r] = 2 * i
        perm[qd * 32 + 16 + r] = 2 * i + 1
    return perm


def _rope_tables():
    """cos_dup/sin_signed [128, S]: per-partition rope tables matching the
    de-interleaved layout (pattern repeats every 64 partitions)."""
    inv_freq = 1.0 / (THETA ** (np.arange(0, HD, 2, dtype=np.float64) / HD))  # [32]
    pos = np.arange(S, dtype=np.float64)
    ang = pos[None, :] * inv_freq[:, None]  # [32, S]
    cos = np.cos(ang)
    sin = np.sin(ang)
    cos_dup = np.zeros((128, S), dtype=np.float32)
    sin_signed = np.zeros((128, S), dtype=np.float32)
    for p in range(128):
        d = p % HD
        qd, r0 = divmod(d, 32)
        if r0 < 16:
            i = qd * 16 + r0
            cos_dup[p] = cos[i]
            sin_signed[p] = -sin[i]
        else:
            i = qd * 16 + (r0 - 16)
            cos_dup[p] = cos[i]
            sin_signed[p] = sin[i]
    return cos_dup, sin_signed


def _build_program():
    import concourse.bass as bass
    from concourse import bacc, mybir
    import concourse.tile as tile

    f32 = mybir.dt.float32
    f32r = mybir.dt.float32r
    bf16 = mybir.dt.bfloat16
    ADD = mybir.AluOpType.add
    MULT = mybir.AluOpType.mult
    EXP = mybir.ActivationFunctionType.Exp
    SWAP16 = [(j + 16) % 32 for j in range(32)]
    DK = D // 128  # 8 contraction chunks

    nc = bacc.Bacc("TRN2", target_bir_lowering=False, debug=False)
    xT = nc.dram_tensor("xT", [D, S], bf16, kind="ExternalInput").ap()
    wq = nc.dram_tensor("wq", [D, DC], bf16, kind="ExternalInput").ap()
    wk = nc.dram_tensor("wk", [D, DC], bf16, kind="ExternalInput").ap()
    wv = nc.dram_tensor("wv", [D, DC], bf16, kind="ExternalInput").ap()
    wo = nc.dram_tensor("wo", [DC, D], bf16, kind="ExternalInput").ap()
    cosd = nc.dram_tensor("cosd", [128, S], bf16, kind="ExternalInput").ap()
    sind = nc.dram_tensor("sind", [128, S], bf16, kind="ExternalInput").ap()
    tri = nc.dram_tensor("tri", [KC, KC], f32, kind="ExternalInput").ap()
    sel = nc.dram_tensor("sel", [128, GQ * 128], bf16,
                         kind="ExternalInput").ap()
    vone = nc.dram_tensor("vone", [128, (S // KC) * HPC], bf16,
                          kind="ExternalInput").ap()
    outT = nc.dram_tensor("outT", [D, S], bf16, kind="ExternalOutput").ap()

    with tile.TileContext(nc) as tc:
        with tc.tile_pool(name="const", bufs=1) as const:
            cos_sb = const.tile([128, S], bf16)
            sin_sb = const.tile([128, S], bf16)
            tri_sb = const.tile([KC, KC], f32)
            wq_sb = const.tile([128, DK, DC], bf16)
            wk_sb = const.tile([128, DK, DC], bf16)
            wv_sb = const.tile([128, DK, DC], bf16)
            wo_sb = const.tile([128, GQ, D], bf16)
            xT_sb = const.tile([128, DK, S], bf16)
            qT_sb = const.tile([128, GQ, S], bf16)
            kT_sb = const.tile([128, GQ, S], bf16)
            vaug_sb = const.tile([128, S // KC, HPC * (HD + 1)], bf16)
            oT_sb = const.tile([128, GQ, S], bf16)
            sums_sb = const.tile([128, S], f32)
            recip_sb = const.tile([128, S], f32)
            recip_bf = const.tile([128, S], bf16)
            sel_sb = const.tile([128, GQ, 128], bf16)

            # DMA order tuned so phase 1 can start ~5us in.
            xTr = xT.rearrange("(o p) n -> p o n", p=128)
            nc.sync.dma_start(wq_sb, wq.rearrange("(o p) n -> p o n", p=128))
            nc.sync.dma_start(xT_sb[:, :, 0:QT], xTr[:, :, 0:QT])
            nc.sync.dma_start(wk_sb, wk.rearrange("(o p) n -> p o n", p=128))
            nc.sync.dma_start(cos_sb[:, 0:QT], cosd[:, 0:QT])
            nc.sync.dma_start(sin_sb[:, 0:QT], sind[:, 0:QT])
            nc.sync.dma_start(wv_sb, wv.rearrange("(o p) n -> p o n", p=128))
            nc.sync.dma_start(tri_sb, tri)
            nc.sync.dma_start(
                vaug_sb[:, :, HD::(HD + 1)],
                vone.rearrange("p (a b) -> p a b", a=S // KC))
            nc.sync.dma_start(
                sel_sb, sel.rearrange("p (c n) -> p c n", c=GQ))
            for qt in range(1, NQT):
                q0 = qt * QT
                nc.sync.dma_start(xT_sb[:, :, q0:q0 + QT], xTr[:, :, q0:q0 + QT])
                nc.sync.dma_start(cos_sb[:, q0:q0 + QT], cosd[:, q0:q0 + QT])
                nc.sync.dma_start(sin_sb[:, q0:q0 + QT], sind[:, q0:q0 + QT])
            nc.sync.dma_start(wo_sb, wo.rearrange("(o p) n -> p o n", p=128))

            # ---- Phase 1: q/k/v projections + rope (PE warm, scalar idle) ----
            with tc.tile_pool(name="p1", bufs=2, space="PSUM") as p1, \
                 tc.tile_pool(name="tmp1", bufs=3) as tmp1:
                # sums_sb init to 1.0 via ALU (memset >1 column miscompiles):
                # garbage lanes must stay finite-nonzero for the batched recip.
                # On gpsimd: the vector engine is phase 1's bottleneck.
                for qt in range(NQT):
                    q0 = qt * QT
                    nc.gpsimd.tensor_scalar(
                        sums_sb[:, q0:q0 + QT], cos_sb[:, q0:q0 + QT],
                        0.0, 1.0, MULT, ADD)

                def rope(ps, dst, q0):
                    # bf16 intermediates: ~2x DVE throughput; psum stays f32
                    shuf = tmp1.tile([128, QT], f32, tag="shuf")
                    nc.vector.stream_shuffle(shuf, ps, SWAP16)
                    m1 = tmp1.tile([128, QT], bf16, tag="m1")
                    nc.vector.tensor_tensor(m1, ps, cos_sb[:, q0:q0 + QT], MULT)
                    m2 = tmp1.tile([128, QT], bf16, tag="m2")
                    nc.vector.tensor_tensor(m2, shuf, sin_sb[:, q0:q0 + QT], MULT)
                    nc.vector.tensor_tensor(dst, m1, m2, ADD)

                import concourse.bass as _b
                for qt in range(NQT):
                    q0 = qt * QT
                    for g in range(GQ):
                        ps_q = p1.tile([128, QT], f32, tag="q")
                        for kc in range(DK):
                            nc.tensor.matmul(
                                ps_q, wq_sb[:, kc, g * 128:(g + 1) * 128],
                                xT_sb[:, kc, q0:q0 + QT],
                                start=(kc == 0), stop=(kc == DK - 1))
                        ps_k = p1.tile([128, QT], f32, tag="k")
                        for kc in range(DK):
                            nc.tensor.matmul(
                                ps_k, wk_sb[:, kc, g * 128:(g + 1) * 128],
                                xT_sb[:, kc, q0:q0 + QT],
                                start=(kc == 0), stop=(kc == DK - 1))
                        rope(ps_q, qT_sb[:, g, q0:q0 + QT], q0)
                        rope(ps_k, kT_sb[:, g, q0:q0 + QT], q0)
                # v-projections last: their PSUM banks + DVE copies drain
                # quickly, so attention's pools start without waiting on rope.
                for qt in range(NQT):
                    q0 = qt * QT
                    for rc in range(QT // KC):
                        r0 = q0 + rc * KC
                        ps_v = p1.tile([128, DC], f32, tag="v", bufs=4)
                        for kc in range(DK):
                            nc.tensor.matmul(
                                ps_v, xT_sb[:, kc, r0:r0 + KC],
                                wv_sb[:, kc, :],
                                start=(kc == 0), stop=(kc == DK - 1))
                        # one strided copy: psum [128,(h d)] -> vaug 65-pitch
                        vdst = vaug_sb[:, r0 // KC, 0:HD]
                        dst3 = _b.AP(tensor=vdst.tensor, offset=vdst.offset,
                                     ap=[list(vdst.ap[0]), [HD + 1, HPC],
                                         [1, HD]])
                        src3 = _b.AP(tensor=ps_v.tensor, offset=ps_v.offset,
                                     ap=[list(ps_v.ap[0]), [HD, HPC],
                                         [1, HD]])
                        nc.vector.tensor_copy(out=dst3, in_=src3)

            # ---- Phase 3: causal flash attention (S^T orientation) ----
            # Scores for the two heads of a group are a row-tiled concurrent
            # pair into one 2-bank PSUM tile, EXPed by a single scalar op.
            # AV uses the ones-row trick (M=65): psum row 64 = softmax sums.
            with tc.tile_pool(name="pss", bufs=2, space="PSUM") as pss, \
                 tc.tile_pool(name="po", bufs=2, space="PSUM") as po, \
                 tc.tile_pool(name="ppr", bufs=5) as ppr:
                # Software-pipelined: AV of chunk i is emitted after the
                # S-pair of chunk i+2, so the PE never queues behind EXP(i)
                # even when the diag-chunk tri-add delays EXP.
                chunks = []
                for g in range(GQ):
                    for qt in range(NQT):
                        nkc = (qt * QT + QT) // KC
                        for kc in range(nkc):
                            chunks.append((g, qt, kc, nkc))

                state = {}  # (g, qt) -> ps_o pair
                pendq = []  # [(g, qt, kc, nkc, probs, qlo)]

                def emit_av(p):
                    g, qt, kc, nkc, probs, qlo = p
                    q0 = qt * QT
                    for a in range(2):
                        h = 2 * g + a
                        nc.tensor.matmul(
                            state[(g, qt)][a][:, qlo:QT],
                            vaug_sb[:, kc, h * (HD + 1):(h + 1) * (HD + 1)],
                            probs[:, a, qlo:QT],
                            start=(kc == 0), stop=(kc == nkc - 1))
                    if kc == nkc - 1:
                        for a in range(2):
                            h = 2 * g + a
                            nc.vector.tensor_copy(
                                out=oT_sb[a * HD:(a + 1) * HD, g, q0:q0 + QT],
                                in_=state[(g, qt)][a][0:HD, :])
                            nc.vector.tensor_copy(
                                out=sums_sb[32 * h:32 * h + 1, q0:q0 + QT],
                                in_=state[(g, qt)][a][HD:HD + 1, :])
                        del state[(g, qt)]
                        if (g, qt) == (GQ - 1, NQT - 2):
                            # all sums except qt=NQT-1 are final: invert the
                            # first 3/4 while the last q-tile is still running
                            nc.vector.reciprocal_approx_fast(
                                recip_sb[:, 0:(NQT - 1) * QT],
                                sums_sb[:, 0:(NQT - 1) * QT])
                            nc.vector.tensor_copy(
                                out=recip_bf[:, 0:(NQT - 1) * QT],
                                in_=recip_sb[:, 0:(NQT - 1) * QT])

                for g, qt, kc, nkc in chunks:
                    q0 = qt * QT
                    k0 = kc * KC
                    j = k0 - q0
                    qlo = max(0, j)
                    if kc == 0:
                        state[(g, qt)] = [
                            po.tile([HD + 1, QT], f32, tag=f"o{a}",
                                    name=f"ps_o{g}_{qt}_{a}")
                            for a in range(2)]
                    ps_s = pss.tile([128, 2, QT], f32, tag="s")
                    for a in range(2):
                        nc.tensor.matmul(
                            ps_s[:, a, qlo:QT],
                            kT_sb[a * HD:(a + 1) * HD, g, k0:k0 + KC],
                            qT_sb[a * HD:(a + 1) * HD, g, q0 + qlo:q0 + QT],
                            start=True, stop=True)
                    if len(pendq) >= 2:
                        emit_av(pendq.pop(0))
                    if j >= 0:
                        for a in range(2):
                            nc.vector.tensor_tensor(
                                ps_s[:, a, qlo:qlo + KC],
                                ps_s[:, a, qlo:qlo + KC], tri_sb, ADD)
                    probs = ppr.tile([128, 2, QT], bf16, tag="p")
                    nc.scalar.activation(
                        probs[:, :, qlo:QT], ps_s[:, :, qlo:QT], EXP)
                    pendq.append((g, qt, kc, nkc, probs, qlo))
                for p in pendq:
                    emit_av(p)

            # ---- Phase 4+5: normalize and output projection ----
            with tc.tile_pool(name="p5", bufs=4, space="PSUM") as p5, \
                 tc.tile_pool(name="pr", bufs=2, space="PSUM") as pr, \
                 tc.tile_pool(name="p5s", bufs=6) as p5s:
                # last q-tile's reciprocal (first 3/4 were emitted inside the
                # attention loop to hide the latency)
                q3 = (NQT - 1) * QT
                nc.vector.reciprocal_approx_fast(
                    recip_sb[:, q3:S], sums_sb[:, q3:S])
                nc.vector.tensor_copy(
                    out=recip_bf[:, q3:S], in_=recip_sb[:, q3:S])
                for qt in range(NQT):
                    q0 = qt * QT
                    for g in range(GQ):
                        ps_r = pr.tile([128, QT], f32, tag="r")
                        nc.tensor.matmul(ps_r, sel_sb[:, g, :],
                                         recip_bf[:, q0:q0 + QT],
                                         start=True, stop=True)
                        nc.vector.tensor_tensor(
                            oT_sb[:, g, q0:q0 + QT], oT_sb[:, g, q0:q0 + QT],
                            ps_r, MULT)
                for qt in range(NQT):
                    q0 = qt * QT
                    for ec in range(D // 128):
                        ps = p5.tile([128, QT], f32, tag="f")
                        for g in range(GQ):
                            nc.tensor.matmul(
                                ps, wo_sb[:, g, ec * 128:(ec + 1) * 128],
                                oT_sb[:, g, q0:q0 + QT],
                                start=(g == 0), stop=(g == GQ - 1))
                        ob = p5s.tile([128, QT], bf16, tag="ob")
                        nc.scalar.copy(out=ob[:, 0:QT // 2], in_=ps[:, 0:QT // 2])
                        nc.vector.tensor_copy(out=ob[:, QT // 2:QT],
                                              in_=ps[:, QT // 2:QT])
                        nc.sync.dma_start(
                            outT[ec * 128:(ec + 1) * 128, q0:q0 + QT], ob)

    nc.finalize()
    return nc


def kernel(x, wq, wk, wv, wo):
    import ml_dtypes
    from concourse import bass_utils

    if os.environ.get("BASS_TRACE"):
        _install_axon_ntff_hook()

    bf = ml_dtypes.bfloat16
    x = np.asarray(x, dtype=np.float32)
    wq = np.asarray(wq, dtype=np.float32)
    wk = np.asarray(wk, dtype=np.float32)
    wv = np.asarray(wv, dtype=np.float32)
    wo = np.asarray(wo, dtype=np.float32)

    # Host prep: weight slicing + rope column permutation + tables.
    perm_l = _rope_perm_local()
    perm = np.concatenate([h * HD + perm_l for h in range(NH)])  # [D]
    scale = 1.0 / np.sqrt(HD)
    wq_p = np.ascontiguousarray(wq[:, perm] * scale)
    wk_p = np.ascontiguousarray(wk[:, perm])
    cos_dup, sin_signed = _rope_tables()
    cos_dup = cos_dup.astype(bf)
    sin_signed = sin_signed.astype(bf)
    kl = np.arange(KC)[:, None]
    ql = np.arange(KC)[None, :]
    tri = np.where(ql >= kl, 0.0, MASKVAL).astype(np.float32)

    # sel[p_src, g*128 + p_dst] = 1 iff p_src == 32 * (2g + p_dst//64):
    # broadcast head (2g + p_dst//64)'s recip row onto all its 64 dims.
    sel = np.zeros((128, GQ, 128), dtype=np.float32)
    for g in range(GQ):
        for a in range(2):
            sel[32 * (2 * g + a), g, a * HD:(a + 1) * HD] = 1.0
    sel = np.ascontiguousarray(sel.reshape(128, GQ * 128).astype(bf))

    xTs = [np.ascontiguousarray(x[b].T.astype(bf)) for b in range(B)]

    in_maps = []
    for i in range(NCORES):
        b, g = divmod(i, HPC)
        cs = slice(g * DC, (g + 1) * DC)
        in_maps.append({
            "xT": xTs[b],
            "wq": np.ascontiguousarray(wq_p[:, cs].astype(bf)),
            "wk": np.ascontiguousarray(wk_p[:, cs].astype(bf)),
            "wv": np.ascontiguousarray(wv[:, cs].astype(bf)),
            "wo": np.ascontiguousarray(wo[cs, :].astype(bf)),
            "cosd": cos_dup,
            "sind": sin_signed,
            "tri": tri,
            "sel": sel,
            "vone": np.ones((128, (S // KC) * HPC), dtype=bf),
        })

    if "nc" not in _CACHE:
        _CACHE["nc"] = _build_program()
    nc = _CACHE["nc"]

    res = bass_utils.run_bass_kernel_spmd(nc, in_maps, core_ids=list(range(NCORES)))
    _CACHE["last_exec_time_ns"] = res.exec_time_ns
    _CACHE["last_res"] = res

    out = np.empty((B, S, D), dtype=np.float32)
    for b in range(B):
        acc = res.results[b * HPC]["outT"].astype(np.float32)
        for g in range(1, HPC):
            acc += res.results[b * HPC + g]["outT"].astype(np.float32)
        out[b] = acc.T
    return out


# revision 56
# speedup vs baseline: 1.1443x; 1.1443x over previous
"""Trainium2 Bass kernel for causal multi-head attention with interleaved RoPE.

Problem: B=2, S=2048, D=1024, 16 heads x 64 dims, causal, rope theta=1e4.

Sharding (8 cores): 2-way batch x 4-way head tensor-parallel.
  core i: batch b = i // 4, head group g = i % 4 (heads 4g..4g+3, dims 256).
  Each core computes q/k/v for its heads from x[b], runs causal flash
  attention, and produces a partial output projection outT [D, S].  Host
  sums the 4 partials per batch and transposes.

Performance design (v2):
  - Inputs in bf16 (halves HBM traffic); x streamed in 512-column chunks so
    projections start ~5us in instead of waiting 46us for the full load.
  - Scores: the two heads of a 128-partition group are computed as a
    row-tiled matmul pair (tile_position (0,0)/(64,0)) so both K=64
    contractions run concurrently in the PE array.
  - Both heads' score chunks live in one [128, 2, 512] PSUM tile (2 banks)
    and are EXPed by a single scalar activation -> fewer scalar instructions
    (scalar engine is the bottleneck of the attention phase; concurrent
    scalar activity also throttles the PE to ~1.2GHz, so PE work per chunk
    is halved via pairing).
  - AV: col-tiled pair (tile_position (0,0)/(0,64)) into two separate PSUM
    banks (separate banks because a matmul with start=True clears the
    has_written bits of its whole bank).
  - Softmax denominators: probs are accumulated on the Vector engine into
    sacc, reduced with M=1 ones-matmuls, inverted with one
    reciprocal_approx_fast, and broadcast back with a selection matmul --
    no DRAM round trip.
  - Output projection per q-tile right after normalize, overlapping DMA out.
"""

import os
import sys

sys.path.insert(0, "/opt/trn_rl_repo")

import numpy as np

B = 2
S = 2048
D = 1024
NH = 16
HD = 64
THETA = 10000.0
NCORES = 8
HPC = 4  # heads per core
DC = HPC * HD  # 256 dims per core
GQ = 2  # 128-partition head groups per core
QT = 512  # query tile (free dim)
NQT = S // QT
KC = 128  # key chunk (partition dim)
MASKVAL = -60.0

_CACHE = {}


def _install_axon_ntff_hook():
    """Register antenv.axon_hooks so trace=True (BASS_TRACE=1) works."""
    import types

    if "antenv.axon_hooks" in sys.modules:
        return
    m = types.ModuleType("antenv.axon_hooks")
    _hook = [None]
    m.set_axon_ntff_profile_hook = lambda h: _hook.__setitem__(0, h)
    m.get_axon_ntff_profile_hook = lambda: _hook[0]
    sys.modules["antenv.axon_hooks"] = m
    try:
        import antenv

        antenv.axon_hooks = m
        from trn_agent_boot.trn_boot import _ntff_profile_via_ctypes

        hook = _ntff_profile_via_ctypes("/opt/axon/libaxon_pjrt.so")
        if hook is not None:
            m.set_axon_ntff_profile_hook(hook)
    except Exception:
        pass


def _rope_perm_local():
    """Permutation of one head's 64 dims: original interleaved pair (2i, 2i+1)
    -> t0 at quadrant*32 + (i%16), t1 at quadrant*32 + 16 + (i%16), with
    quadrant = i // 16.  Returns perm such that new[j] = old[perm[j]]."""
    perm = np.zeros(HD, dtype=np.int64)
    for i in range(HD // 2):
        qd, r = divmod(i, 16)
        perm[qd * 32 + r] = 2 * i
        perm[qd * 32 + 16 + r] = 2 * i + 1
    return perm


def _rope_tables():
    """cos_dup/sin_signed [128, S]: per-partition rope tables matching the
    de-interleaved layout (pattern repeats every 64 partitions)."""
    inv_freq = 1.0 / (THETA ** (np.arange(0, HD, 2, dtype=np.float64) / HD))  # [32]
    pos = np.arange(S, dtype=np.float64)
    ang = pos[None, :] * inv_freq[:, None]  # [32, S]
    cos = np.cos(ang)
    sin = np.sin(ang)
    cos_dup = np.zeros((128, S), dtype=np.float32)
    sin_signed = np.zeros((128, S), dtype=np.float32)
    for p in range(128):
        d = p % HD
        qd, r0 = divmod(d, 32)
        if r0 < 16:
            i = qd * 16 + r0
            cos_dup[p] = cos[i]
            sin_signed[p] = -sin[i]
        else:
            i = qd * 16 + (r0 - 16)
            cos_dup[p] = cos[i]
            sin_signed[p] = sin[i]
    return cos_dup, sin_signed


def _build_program():
    import concourse.bass as bass
    from concourse import bacc, mybir
    import concourse.tile as tile

    f32 = mybir.dt.float32
    f32r = mybir.dt.float32r
    bf16 = mybir.dt.bfloat16
    ADD = mybir.AluOpType.add
    MULT = mybir.AluOpType.mult
    EXP = mybir.ActivationFunctionType.Exp
    SWAP16 = [(j + 16) % 32 for j in range(32)]
    DK = D // 128  # 8 contraction chunks

    nc = bacc.Bacc("TRN2", target_bir_lowering=False, debug=False)
    xT = nc.dram_tensor("xT", [D, S], bf16, kind="ExternalInput").ap()
    wq = nc.dram_tensor("wq", [D, DC], bf16, kind="ExternalInput").ap()
    wk = nc.dram_tensor("wk", [D, DC], bf16, kind="ExternalInput").ap()
    wv = nc.dram_tensor("wv", [D, DC], bf16, kind="ExternalInput").ap()
    wo = nc.dram_tensor("wo", [DC, D], bf16, kind="ExternalInput").ap()
    cosd = nc.dram_tensor("cosd", [128, S], bf16, kind="ExternalInput").ap()
    sind = nc.dram_tensor("sind", [128, S], bf16, kind="ExternalInput").ap()
    tri = nc.dram_tensor("tri", [KC, KC], f32, kind="ExternalInput").ap()
    sel = nc.dram_tensor("sel", [128, GQ * 128], bf16,
                         kind="ExternalInput").ap()
    vone = nc.dram_tensor("vone", [128, (S // KC) * HPC], bf16,
                          kind="ExternalInput").ap()
    outT = nc.dram_tensor("outT", [D, S], bf16, kind="ExternalOutput").ap()

    with tile.TileContext(nc) as tc:
        with tc.tile_pool(name="const", bufs=1) as const:
            cos_sb = const.tile([128, S], bf16)
            sin_sb = const.tile([128, S], bf16)
            tri_sb = const.tile([KC, KC], f32)
            wq_sb = const.tile([128, DK, DC], bf16)
            wk_sb = const.tile([128, DK, DC], bf16)
            wv_sb = const.tile([128, DK, DC], bf16)
            wo_sb = const.tile([128, GQ, D], bf16)
            xT_sb = const.tile([128, DK, S], bf16)
            qT_sb = const.tile([128, GQ, S], bf16)
            kT_sb = const.tile([128, GQ, S], bf16)
            vaug_sb = const.tile([128, S // KC, HPC * (HD + 1)], bf16)
            oT_sb = const.tile([128, GQ, S], bf16)
            sums_sb = const.tile([128, S], f32)
            recip_sb = const.tile([128, S], f32)
            recip_bf = const.tile([128, S], bf16)
            sel_sb = const.tile([128, GQ, 128], bf16)

            # DMA order tuned so phase 1 can start ~5us in.
            xTr = xT.rearrange("(o p) n -> p o n", p=128)
            nc.sync.dma_start(wq_sb, wq.rearrange("(o p) n -> p o n", p=128))
            nc.sync.dma_start(xT_sb[:, :, 0:QT], xTr[:, :, 0:QT])
            nc.sync.dma_start(wk_sb, wk.rearrange("(o p) n -> p o n", p=128))
            nc.sync.dma_start(cos_sb[:, 0:QT], cosd[:, 0:QT])
            nc.sync.dma_start(sin_sb[:, 0:QT], sind[:, 0:QT])
            nc.sync.dma_start(wv_sb, wv.rearrange("(o p) n -> p o n", p=128))
            nc.sync.dma_start(tri_sb, tri)
            nc.sync.dma_start(
                vaug_sb[:, :, HD::(HD + 1)],
                vone.rearrange("p (a b) -> p a b", a=S // KC))
            nc.sync.dma_start(
                sel_sb, sel.rearrange("p (c n) -> p c n", c=GQ))
            for qt in range(1, NQT):
                q0 = qt * QT
                nc.sync.dma_start(xT_sb[:, :, q0:q0 + QT], xTr[:, :, q0:q0 + QT])
                nc.sync.dma_start(cos_sb[:, q0:q0 + QT], cosd[:, q0:q0 + QT])
                nc.sync.dma_start(sin_sb[:, q0:q0 + QT], sind[:, q0:q0 + QT])
            nc.sync.dma_start(wo_sb, wo.rearrange("(o p) n -> p o n", p=128))

            # ---- Phase 1: q/k/v projections + rope (PE warm, scalar idle) ----
            with tc.tile_pool(name="p1", bufs=2, space="PSUM") as p1, \
                 tc.tile_pool(name="tmp1", bufs=3) as tmp1:
                # sums_sb init to 1.0 via ALU (memset >1 column miscompiles):
                # garbage lanes must stay finite-nonzero for the batched recip.
                # On gpsimd: the vector engine is phase 1's bottleneck.
                for qt in range(NQT):
                    q0 = qt * QT
                    nc.vector.tensor_scalar(
                        sums_sb[:, q0:q0 + QT], cos_sb[:, q0:q0 + QT],
                        0.0, 1.0, MULT, ADD)

                def rope(ps, dst, q0):
                    # bf16 intermediates: ~2x DVE throughput; psum stays f32
                    shuf = tmp1.tile([128, QT], f32, tag="shuf")
                    nc.vector.stream_shuffle(shuf, ps, SWAP16)
                    m1 = tmp1.tile([128, QT], f32, tag="m1")
                    nc.vector.tensor_tensor(m1, ps, cos_sb[:, q0:q0 + QT], MULT)
                    m2 = tmp1.tile([128, QT], f32, tag="m2")
                    nc.vector.tensor_tensor(m2, shuf, sin_sb[:, q0:q0 + QT], MULT)
                    nc.vector.tensor_tensor(dst, m1, m2, ADD)

                import concourse.bass as _b
                for qt in range(NQT):
                    q0 = qt * QT
                    for g in range(GQ):
                        ps_q = p1.tile([128, QT], f32, tag="q")
                        for kc in range(DK):
                            nc.tensor.matmul(
                                ps_q, wq_sb[:, kc, g * 128:(g + 1) * 128],
                                xT_sb[:, kc, q0:q0 + QT],
                                start=(kc == 0), stop=(kc == DK - 1))
                        ps_k = p1.tile([128, QT], f32, tag="k")
                        for kc in range(DK):
                            nc.tensor.matmul(
                                ps_k, wk_sb[:, kc, g * 128:(g + 1) * 128],
                                xT_sb[:, kc, q0:q0 + QT],
                                start=(kc == 0), stop=(kc == DK - 1))
                        rope(ps_q, qT_sb[:, g, q0:q0 + QT], q0)
                        rope(ps_k, kT_sb[:, g, q0:q0 + QT], q0)
                # v-projections last: their PSUM banks + DVE copies drain
                # quickly, so attention's pools start without waiting on rope.
                for qt in range(NQT):
                    q0 = qt * QT
                    for rc in range(QT // KC):
                        r0 = q0 + rc * KC
                        ps_v = p1.tile([128, DC], f32, tag="v", bufs=4)
                        for kc in range(DK):
                            nc.tensor.matmul(
                                ps_v, xT_sb[:, kc, r0:r0 + KC],
                                wv_sb[:, kc, :],
                                start=(kc == 0), stop=(kc == DK - 1))
                        # one strided copy: psum [128,(h d)] -> vaug 65-pitch
                        vdst = vaug_sb[:, r0 // KC, 0:HD]
                        dst3 = _b.AP(tensor=vdst.tensor, offset=vdst.offset,
                                     ap=[list(vdst.ap[0]), [HD + 1, HPC],
                                         [1, HD]])
                        src3 = _b.AP(tensor=ps_v.tensor, offset=ps_v.offset,
                                     ap=[list(ps_v.ap[0]), [HD, HPC],
                                         [1, HD]])
                        nc.vector.tensor_copy(out=dst3, in_=src3)

            # ---- Phase 3: causal flash attention (S^T orientation) ----
            # Scores for the two heads of a group are a row-tiled concurrent
            # pair into one 2-bank PSUM tile, EXPed by a single scalar op.
            # AV uses the ones-row trick (M=65): psum row 64 = softmax sums.
            with tc.tile_pool(name="pss", bufs=2, space="PSUM") as pss, \
                 tc.tile_pool(name="po", bufs=2, space="PSUM") as po, \
                 tc.tile_pool(name="ppr", bufs=5) as ppr:
                # Software-pipelined: AV of chunk i is emitted after the
                # S-pair of chunk i+2, so the PE never queues behind EXP(i)
                # even when the diag-chunk tri-add delays EXP.
                chunks = []
                for g in range(GQ):
                    for qt in range(NQT):
                        nkc = (qt * QT + QT) // KC
                        for kc in range(nkc):
                            chunks.append((g, qt, kc, nkc))

                state = {}  # (g, qt) -> ps_o pair
                pendq = []  # [(g, qt, kc, nkc, probs, qlo)]

                def emit_av(p):
                    g, qt, kc, nkc, probs, qlo = p
                    q0 = qt * QT
                    for a in range(2):
                        h = 2 * g + a
                        nc.tensor.matmul(
                            state[(g, qt)][a][:, qlo:QT],
                            vaug_sb[:, kc, h * (HD + 1):(h + 1) * (HD + 1)],
                            probs[:, a, qlo:QT],
                            start=(kc == 0), stop=(kc == nkc - 1))
                    if kc == nkc - 1:
                        for a in range(2):
                            h = 2 * g + a
                            nc.vector.tensor_copy(
                                out=oT_sb[a * HD:(a + 1) * HD, g, q0:q0 + QT],
                                in_=state[(g, qt)][a][0:HD, :])
                            nc.vector.tensor_copy(
                                out=sums_sb[32 * h:32 * h + 1, q0:q0 + QT],
                                in_=state[(g, qt)][a][HD:HD + 1, :])
                        del state[(g, qt)]
                        if (g, qt) == (GQ - 1, NQT - 2):
                            # all sums except qt=NQT-1 are final: invert the
                            # first 3/4 while the last q-tile is still running
                            nc.vector.reciprocal_approx_fast(
                                recip_sb[:, 0:(NQT - 1) * QT],
                                sums_sb[:, 0:(NQT - 1) * QT])
                            nc.vector.tensor_copy(
                                out=recip_bf[:, 0:(NQT - 1) * QT],
                                in_=recip_sb[:, 0:(NQT - 1) * QT])

                for g, qt, kc, nkc in chunks:
                    q0 = qt * QT
                    k0 = kc * KC
                    j = k0 - q0
                    qlo = max(0, j)
                    if kc == 0:
                        state[(g, qt)] = [
                            po.tile([HD + 1, QT], f32, tag=f"o{a}",
                                    name=f"ps_o{g}_{qt}_{a}")
                            for a in range(2)]
                    ps_s = pss.tile([128, 2, QT], f32, tag="s")
                    for a in range(2):
                        nc.tensor.matmul(
                            ps_s[:, a, qlo:QT],
                            kT_sb[a * HD:(a + 1) * HD, g, k0:k0 + KC],
                            qT_sb[a * HD:(a + 1) * HD, g, q0 + qlo:q0 + QT],
                            start=True, stop=True)
                    if len(pendq) >= 2:
                        emit_av(pendq.pop(0))
                    if j >= 0:
                        for a in range(2):
                            nc.vector.tensor_tensor(
                                ps_s[:, a, qlo:qlo + KC],
                                ps_s[:, a, qlo:qlo + KC], tri_sb, ADD)
                    probs = ppr.tile([128, 2, QT], bf16, tag="p")
                    nc.scalar.activation(
                        probs[:, :, qlo:QT], ps_s[:, :, qlo:QT], EXP)
                    pendq.append((g, qt, kc, nkc, probs, qlo))
                for p in pendq:
                    emit_av(p)

            # ---- Phase 4+5: normalize and output projection ----
            with tc.tile_pool(name="p5", bufs=4, space="PSUM") as p5, \
                 tc.tile_pool(name="pr", bufs=2, space="PSUM") as pr, \
                 tc.tile_pool(name="p5s", bufs=6) as p5s:
                # last q-tile's reciprocal (first 3/4 were emitted inside the
                # attention loop to hide the latency)
                q3 = (NQT - 1) * QT
                nc.vector.reciprocal_approx_fast(
                    recip_sb[:, q3:S], sums_sb[:, q3:S])
                nc.vector.tensor_copy(
                    out=recip_bf[:, q3:S], in_=recip_sb[:, q3:S])
                for qt in range(NQT):
                    q0 = qt * QT
                    for g in range(GQ):
                        ps_r = pr.tile([128, QT], f32, tag="r")
                        nc.tensor.matmul(ps_r, sel_sb[:, g, :],
                                         recip_bf[:, q0:q0 + QT],
                                         start=True, stop=True)
                        nc.vector.tensor_tensor(
                            oT_sb[:, g, q0:q0 + QT], oT_sb[:, g, q0:q0 + QT],
                            ps_r, MULT)
                for qt in range(NQT):
                    q0 = qt * QT
                    for ec in range(D // 128):
                        ps = p5.tile([128, QT], f32, tag="f")
                        for g in range(GQ):
                            nc.tensor.matmul(
                                ps, wo_sb[:, g, ec * 128:(ec + 1) * 128],
                                oT_sb[:, g, q0:q0 + QT],
                                start=(g == 0), stop=(g == GQ - 1))
                        ob = p5s.tile([128, QT], bf16, tag="ob")
                        nc.scalar.copy(out=ob[:, 0:QT // 2], in_=ps[:, 0:QT // 2])
                        nc.vector.tensor_copy(out=ob[:, QT // 2:QT],
                                              in_=ps[:, QT // 2:QT])
                        nc.sync.dma_start(
                            outT[ec * 128:(ec + 1) * 128, q0:q0 + QT], ob)

    nc.finalize()
    return nc


def kernel(x, wq, wk, wv, wo):
    import ml_dtypes
    from concourse import bass_utils

    if os.environ.get("BASS_TRACE"):
        _install_axon_ntff_hook()

    bf = ml_dtypes.bfloat16
    x = np.asarray(x, dtype=np.float32)
    wq = np.asarray(wq, dtype=np.float32)
    wk = np.asarray(wk, dtype=np.float32)
    wv = np.asarray(wv, dtype=np.float32)
    wo = np.asarray(wo, dtype=np.float32)

    # Host prep: weight slicing + rope column permutation + tables.
    perm_l = _rope_perm_local()
    perm = np.concatenate([h * HD + perm_l for h in range(NH)])  # [D]
    scale = 1.0 / np.sqrt(HD)
    wq_p = np.ascontiguousarray(wq[:, perm] * scale)
    wk_p = np.ascontiguousarray(wk[:, perm])
    cos_dup, sin_signed = _rope_tables()
    cos_dup = cos_dup.astype(bf)
    sin_signed = sin_signed.astype(bf)
    kl = np.arange(KC)[:, None]
    ql = np.arange(KC)[None, :]
    tri = np.where(ql >= kl, 0.0, MASKVAL).astype(np.float32)

    # sel[p_src, g*128 + p_dst] = 1 iff p_src == 32 * (2g + p_dst//64):
    # broadcast head (2g + p_dst//64)'s recip row onto all its 64 dims.
    sel = np.zeros((128, GQ, 128), dtype=np.float32)
    for g in range(GQ):
        for a in range(2):
            sel[32 * (2 * g + a), g, a * HD:(a + 1) * HD] = 1.0
    sel = np.ascontiguousarray(sel.reshape(128, GQ * 128).astype(bf))

    xTs = [np.ascontiguousarray(x[b].T.astype(bf)) for b in range(B)]

    in_maps = []
    for i in range(NCORES):
        b, g = divmod(i, HPC)
        cs = slice(g * DC, (g + 1) * DC)
        in_maps.append({
            "xT": xTs[b],
            "wq": np.ascontiguousarray(wq_p[:, cs].astype(bf)),
            "wk": np.ascontiguousarray(wk_p[:, cs].astype(bf)),
            "wv": np.ascontiguousarray(wv[:, cs].astype(bf)),
            "wo": np.ascontiguousarray(wo[cs, :].astype(bf)),
            "cosd": cos_dup,
            "sind": sin_signed,
            "tri": tri,
            "sel": sel,
            "vone": np.ones((128, (S // KC) * HPC), dtype=bf),
        })

    if "nc" not in _CACHE:
        _CACHE["nc"] = _build_program()
    nc = _CACHE["nc"]

    res = bass_utils.run_bass_kernel_spmd(nc, in_maps, core_ids=list(range(NCORES)))
    _CACHE["last_exec_time_ns"] = res.exec_time_ns
    _CACHE["last_res"] = res

    out = np.empty((B, S, D), dtype=np.float32)
    for b in range(B):
        acc = res.results[b * HPC]["outT"].astype(np.float32)
        for g in range(1, HPC):
            acc += res.results[b * HPC + g]["outT"].astype(np.float32)
        out[b] = acc.T
    return out


# revision 57
# speedup vs baseline: 1.1927x; 1.0423x over previous
"""Trainium2 Bass kernel for causal multi-head attention with interleaved RoPE.

Problem: B=2, S=2048, D=1024, 16 heads x 64 dims, causal, rope theta=1e4.

Sharding (8 cores): 2-way batch x 4-way head tensor-parallel.
  core i: batch b = i // 4, head group g = i % 4 (heads 4g..4g+3, dims 256).
  Each core computes q/k/v for its heads from x[b], runs causal flash
  attention, and produces a partial output projection outT [D, S].  Host
  sums the 4 partials per batch and transposes.

Performance design (v2):
  - Inputs in bf16 (halves HBM traffic); x streamed in 512-column chunks so
    projections start ~5us in instead of waiting 46us for the full load.
  - Scores: the two heads of a 128-partition group are computed as a
    row-tiled matmul pair (tile_position (0,0)/(64,0)) so both K=64
    contractions run concurrently in the PE array.
  - Both heads' score chunks live in one [128, 2, 512] PSUM tile (2 banks)
    and are EXPed by a single scalar activation -> fewer scalar instructions
    (scalar engine is the bottleneck of the attention phase; concurrent
    scalar activity also throttles the PE to ~1.2GHz, so PE work per chunk
    is halved via pairing).
  - AV: col-tiled pair (tile_position (0,0)/(0,64)) into two separate PSUM
    banks (separate banks because a matmul with start=True clears the
    has_written bits of its whole bank).
  - Softmax denominators: probs are accumulated on the Vector engine into
    sacc, reduced with M=1 ones-matmuls, inverted with one
    reciprocal_approx_fast, and broadcast back with a selection matmul --
    no DRAM round trip.
  - Output projection per q-tile right after normalize, overlapping DMA out.
"""

import os
import sys

sys.path.insert(0, "/opt/trn_rl_repo")

import numpy as np

B = 2
S = 2048
D = 1024
NH = 16
HD = 64
THETA = 10000.0
NCORES = 8
HPC = 4  # heads per core
DC = HPC * HD  # 256 dims per core
GQ = 2  # 128-partition head groups per core
QT = 512  # query tile (free dim)
NQT = S // QT
KC = 128  # key chunk (partition dim)
MASKVAL = -60.0

_CACHE = {}


def _install_axon_ntff_hook():
    """Register antenv.axon_hooks so trace=True (BASS_TRACE=1) works."""
    import types

    if "antenv.axon_hooks" in sys.modules:
        return
    m = types.ModuleType("antenv.axon_hooks")
    _hook = [None]
    m.set_axon_ntff_profile_hook = lambda h: _hook.__setitem__(0, h)
    m.get_axon_ntff_profile_hook = lambda: _hook[0]
    sys.modules["antenv.axon_hooks"] = m
    try:
        import antenv

        antenv.axon_hooks = m
        from trn_agent_boot.trn_boot import _ntff_profile_via_ctypes

        hook = _ntff_profile_via_ctypes("/opt/axon/libaxon_pjrt.so")
        if hook is not None:
            m.set_axon_ntff_profile_hook(hook)
    except Exception:
        pass


def _rope_perm_local():
    """Permutation of one head's 64 dims: original interleaved pair (2i, 2i+1)
    -> t0 at quadrant*32 + (i%16), t1 at quadrant*32 + 16 + (i%16), with
    quadrant = i // 16.  Returns perm such that new[j] = old[perm[j]]."""
    perm = np.zeros(HD, dtype=np.int64)
    for i in range(HD // 2):
        qd, r = divmod(i, 16)
        perm[qd * 32 + r] = 2 * i
        perm[qd * 32 + 16 + r] = 2 * i + 1
    return perm


def _rope_tables():
    """cos_dup/sin_signed [128, S]: per-partition rope tables matching the
    de-interleaved layout (pattern repeats every 64 partitions)."""
    inv_freq = 1.0 / (THETA ** (np.arange(0, HD, 2, dtype=np.float64) / HD))  # [32]
    pos = np.arange(S, dtype=np.float64)
    ang = pos[None, :] * inv_freq[:, None]  # [32, S]
    cos = np.cos(ang)
    sin = np.sin(ang)
    cos_dup = np.zeros((128, S), dtype=np.float32)
    sin_signed = np.zeros((128, S), dtype=np.float32)
    for p in range(128):
        d = p % HD
        qd, r0 = divmod(d, 32)
        if r0 < 16:
            i = qd * 16 + r0
            cos_dup[p] = cos[i]
            sin_signed[p] = -sin[i]
        else:
            i = qd * 16 + (r0 - 16)
            cos_dup[p] = cos[i]
            sin_signed[p] = sin[i]
    return cos_dup, sin_signed


def _build_program():
    import concourse.bass as bass
    from concourse import bacc, mybir
    import concourse.tile as tile

    f32 = mybir.dt.float32
    f32r = mybir.dt.float32r
    bf16 = mybir.dt.bfloat16
    ADD = mybir.AluOpType.add
    MULT = mybir.AluOpType.mult
    EXP = mybir.ActivationFunctionType.Exp
    SWAP16 = [(j + 16) % 32 for j in range(32)]
    DK = D // 128  # 8 contraction chunks

    nc = bacc.Bacc("TRN2", target_bir_lowering=False, debug=False)
    xT = nc.dram_tensor("xT", [D, S], bf16, kind="ExternalInput").ap()
    wq = nc.dram_tensor("wq", [D, DC], bf16, kind="ExternalInput").ap()
    wk = nc.dram_tensor("wk", [D, DC], bf16, kind="ExternalInput").ap()
    wv = nc.dram_tensor("wv", [D, DC], bf16, kind="ExternalInput").ap()
    wo = nc.dram_tensor("wo", [DC, D], bf16, kind="ExternalInput").ap()
    cosd = nc.dram_tensor("cosd", [128, S], bf16, kind="ExternalInput").ap()
    sind = nc.dram_tensor("sind", [128, S], bf16, kind="ExternalInput").ap()
    tri = nc.dram_tensor("tri", [KC, KC], f32, kind="ExternalInput").ap()
    sel = nc.dram_tensor("sel", [128, GQ * 128], bf16,
                         kind="ExternalInput").ap()
    vone = nc.dram_tensor("vone", [128, (S // KC) * HPC], bf16,
                          kind="ExternalInput").ap()
    outT = nc.dram_tensor("outT", [D, S], bf16, kind="ExternalOutput").ap()

    with tile.TileContext(nc) as tc:
        with tc.tile_pool(name="const", bufs=1) as const:
            cos_sb = const.tile([128, S], bf16)
            sin_sb = const.tile([128, S], bf16)
            tri_sb = const.tile([KC, KC], f32)
            wq_sb = const.tile([128, DK, DC], bf16)
            wk_sb = const.tile([128, DK, DC], bf16)
            wv_sb = const.tile([128, DK, DC], bf16)
            wo_sb = const.tile([128, GQ, D], bf16)
            xT_sb = const.tile([128, DK, S], bf16)
            qT_sb = const.tile([128, GQ, S], bf16)
            kT_sb = const.tile([128, GQ, S], bf16)
            vaug_sb = const.tile([128, S // KC, HPC * (HD + 1)], bf16)
            oT_sb = const.tile([128, GQ, S], bf16)
            sums_sb = const.tile([128, S], f32)
            recip_sb = const.tile([128, S], f32)
            recip_bf = const.tile([128, S], bf16)
            sel_sb = const.tile([128, GQ, 128], bf16)

            # DMA order tuned so phase 1 can start ~5us in.
            xTr = xT.rearrange("(o p) n -> p o n", p=128)
            nc.sync.dma_start(wq_sb, wq.rearrange("(o p) n -> p o n", p=128))
            nc.sync.dma_start(xT_sb[:, :, 0:QT], xTr[:, :, 0:QT])
            nc.sync.dma_start(wk_sb, wk.rearrange("(o p) n -> p o n", p=128))
            nc.sync.dma_start(cos_sb[:, 0:QT], cosd[:, 0:QT])
            nc.sync.dma_start(sin_sb[:, 0:QT], sind[:, 0:QT])
            nc.sync.dma_start(wv_sb, wv.rearrange("(o p) n -> p o n", p=128))
            nc.sync.dma_start(tri_sb, tri)
            nc.sync.dma_start(
                vaug_sb[:, :, HD::(HD + 1)],
                vone.rearrange("p (a b) -> p a b", a=S // KC))
            nc.sync.dma_start(
                sel_sb, sel.rearrange("p (c n) -> p c n", c=GQ))
            for qt in range(1, NQT):
                q0 = qt * QT
                nc.sync.dma_start(xT_sb[:, :, q0:q0 + QT], xTr[:, :, q0:q0 + QT])
                nc.sync.dma_start(cos_sb[:, q0:q0 + QT], cosd[:, q0:q0 + QT])
                nc.sync.dma_start(sin_sb[:, q0:q0 + QT], sind[:, q0:q0 + QT])
            nc.sync.dma_start(wo_sb, wo.rearrange("(o p) n -> p o n", p=128))

            # ---- Phase 1: q/k/v projections + rope (PE warm, scalar idle) ----
            with tc.tile_pool(name="p1", bufs=2, space="PSUM") as p1, \
                 tc.tile_pool(name="tmp1", bufs=3) as tmp1:
                # sums_sb init to 1.0 via ALU (memset >1 column miscompiles):
                # garbage lanes must stay finite-nonzero for the batched recip.
                # On gpsimd: the vector engine is phase 1's bottleneck.
                for qt in range(NQT):
                    q0 = qt * QT
                    nc.vector.tensor_scalar(
                        sums_sb[:, q0:q0 + QT], cos_sb[:, q0:q0 + QT],
                        0.0, 1.0, MULT, ADD)

                def rope(ps, dst, q0):
                    # bf16 intermediates: ~2x DVE throughput; psum stays f32
                    shuf = tmp1.tile([128, QT], f32, tag="shuf")
                    nc.vector.stream_shuffle(shuf, ps, SWAP16)
                    m1 = tmp1.tile([128, QT], f32, tag="m1")
                    nc.vector.tensor_tensor(m1, ps, cos_sb[:, q0:q0 + QT], MULT)
                    m2 = tmp1.tile([128, QT], f32, tag="m2")
                    nc.vector.tensor_tensor(m2, shuf, sin_sb[:, q0:q0 + QT], MULT)
                    nc.vector.tensor_tensor(dst, m1, m2, ADD)

                import concourse.bass as _b
                for qt in range(NQT):
                    q0 = qt * QT
                    for g in range(GQ):
                        ps_q = p1.tile([128, QT], f32, tag="q")
                        for kc in range(DK):
                            nc.tensor.matmul(
                                ps_q, wq_sb[:, kc, g * 128:(g + 1) * 128],
                                xT_sb[:, kc, q0:q0 + QT],
                                start=(kc == 0), stop=(kc == DK - 1))
                        ps_k = p1.tile([128, QT], f32, tag="k")
                        for kc in range(DK):
                            nc.tensor.matmul(
                                ps_k, wk_sb[:, kc, g * 128:(g + 1) * 128],
                                xT_sb[:, kc, q0:q0 + QT],
                                start=(kc == 0), stop=(kc == DK - 1))
                        rope(ps_q, qT_sb[:, g, q0:q0 + QT], q0)
                        rope(ps_k, kT_sb[:, g, q0:q0 + QT], q0)
                    for rc in range(QT // KC):
                        r0 = q0 + rc * KC
                        ps_v = p1.tile([128, DC], f32, tag="v", bufs=4)
                        for kc in range(DK):
                            nc.tensor.matmul(
                                ps_v, xT_sb[:, kc, r0:r0 + KC],
                                wv_sb[:, kc, :],
                                start=(kc == 0), stop=(kc == DK - 1))
                        # one strided copy: psum [128,(h d)] -> vaug 65-pitch
                        vdst = vaug_sb[:, r0 // KC, 0:HD]
                        dst3 = _b.AP(tensor=vdst.tensor, offset=vdst.offset,
                                     ap=[list(vdst.ap[0]), [HD + 1, HPC],
                                         [1, HD]])
                        src3 = _b.AP(tensor=ps_v.tensor, offset=ps_v.offset,
                                     ap=[list(ps_v.ap[0]), [HD, HPC],
                                         [1, HD]])
                        nc.vector.tensor_copy(out=dst3, in_=src3)

            # ---- Phase 3: causal flash attention (S^T orientation) ----
            # Scores for the two heads of a group are a row-tiled concurrent
            # pair into one 2-bank PSUM tile, EXPed by a single scalar op.
            # AV uses the ones-row trick (M=65): psum row 64 = softmax sums.
            with tc.tile_pool(name="pss", bufs=2, space="PSUM") as pss, \
                 tc.tile_pool(name="po", bufs=2, space="PSUM") as po, \
                 tc.tile_pool(name="ppr", bufs=5) as ppr:
                # Software-pipelined: AV of chunk i is emitted after the
                # S-pair of chunk i+2, so the PE never queues behind EXP(i)
                # even when the diag-chunk tri-add delays EXP.
                chunks = []
                for g in range(GQ):
                    for qt in range(NQT):
                        nkc = (qt * QT + QT) // KC
                        for kc in range(nkc):
                            chunks.append((g, qt, kc, nkc))

                state = {}  # (g, qt) -> ps_o pair
                pendq = []  # [(g, qt, kc, nkc, probs, qlo)]

                def emit_av(p):
                    g, qt, kc, nkc, probs, qlo = p
                    q0 = qt * QT
                    for a in range(2):
                        h = 2 * g + a
                        nc.tensor.matmul(
                            state[(g, qt)][a][:, qlo:QT],
                            vaug_sb[:, kc, h * (HD + 1):(h + 1) * (HD + 1)],
                            probs[:, a, qlo:QT],
                            start=(kc == 0), stop=(kc == nkc - 1))
                    if kc == nkc - 1:
                        for a in range(2):
                            h = 2 * g + a
                            nc.vector.tensor_copy(
                                out=oT_sb[a * HD:(a + 1) * HD, g, q0:q0 + QT],
                                in_=state[(g, qt)][a][0:HD, :])
                            nc.vector.tensor_copy(
                                out=sums_sb[32 * h:32 * h + 1, q0:q0 + QT],
                                in_=state[(g, qt)][a][HD:HD + 1, :])
                        del state[(g, qt)]
                        if (g, qt) == (GQ - 1, NQT - 2):
                            # all sums except qt=NQT-1 are final: invert the
                            # first 3/4 while the last q-tile is still running
                            nc.vector.reciprocal_approx_fast(
                                recip_sb[:, 0:(NQT - 1) * QT],
                                sums_sb[:, 0:(NQT - 1) * QT])
                            nc.vector.tensor_copy(
                                out=recip_bf[:, 0:(NQT - 1) * QT],
                                in_=recip_sb[:, 0:(NQT - 1) * QT])

                for g, qt, kc, nkc in chunks:
                    q0 = qt * QT
                    k0 = kc * KC
                    j = k0 - q0
                    qlo = max(0, j)
                    if kc == 0:
                        state[(g, qt)] = [
                            po.tile([HD + 1, QT], f32, tag=f"o{a}",
                                    name=f"ps_o{g}_{qt}_{a}")
                            for a in range(2)]
                    ps_s = pss.tile([128, 2, QT], f32, tag="s")
                    for a in range(2):
                        nc.tensor.matmul(
                            ps_s[:, a, qlo:QT],
                            kT_sb[a * HD:(a + 1) * HD, g, k0:k0 + KC],
                            qT_sb[a * HD:(a + 1) * HD, g, q0 + qlo:q0 + QT],
                            start=True, stop=True)
                    if len(pendq) >= 2:
                        emit_av(pendq.pop(0))
                    if j >= 0:
                        for a in range(2):
                            nc.vector.tensor_tensor(
                                ps_s[:, a, qlo:qlo + KC],
                                ps_s[:, a, qlo:qlo + KC], tri_sb, ADD)
                    probs = ppr.tile([128, 2, QT], bf16, tag="p")
                    nc.scalar.activation(
                        probs[:, :, qlo:QT], ps_s[:, :, qlo:QT], EXP)
                    pendq.append((g, qt, kc, nkc, probs, qlo))
                for p in pendq:
                    emit_av(p)

            # ---- Phase 4+5: normalize and output projection ----
            with tc.tile_pool(name="p5", bufs=4, space="PSUM") as p5, \
                 tc.tile_pool(name="pr", bufs=2, space="PSUM") as pr, \
                 tc.tile_pool(name="p5s", bufs=6) as p5s:
                # last q-tile's reciprocal (first 3/4 were emitted inside the
                # attention loop to hide the latency)
                q3 = (NQT - 1) * QT
                nc.vector.reciprocal_approx_fast(
                    recip_sb[:, q3:S], sums_sb[:, q3:S])
                nc.vector.tensor_copy(
                    out=recip_bf[:, q3:S], in_=recip_sb[:, q3:S])
                for qt in range(NQT):
                    q0 = qt * QT
                    for g in range(GQ):
                        ps_r = pr.tile([128, QT], f32, tag="r")
                        nc.tensor.matmul(ps_r, sel_sb[:, g, :],
                                         recip_bf[:, q0:q0 + QT],
                                         start=True, stop=True)
                        nc.vector.tensor_tensor(
                            oT_sb[:, g, q0:q0 + QT], oT_sb[:, g, q0:q0 + QT],
                            ps_r, MULT)
                for qt in range(NQT):
                    q0 = qt * QT
                    for ec in range(D // 128):
                        ps = p5.tile([128, QT], f32, tag="f")
                        for g in range(GQ):
                            nc.tensor.matmul(
                                ps, wo_sb[:, g, ec * 128:(ec + 1) * 128],
                                oT_sb[:, g, q0:q0 + QT],
                                start=(g == 0), stop=(g == GQ - 1))
                        ob = p5s.tile([128, QT], bf16, tag="ob")
                        nc.scalar.copy(out=ob[:, 0:QT // 2], in_=ps[:, 0:QT // 2])
                        nc.vector.tensor_copy(out=ob[:, QT // 2:QT],
                                              in_=ps[:, QT // 2:QT])
                        nc.sync.dma_start(
                            outT[ec * 128:(ec + 1) * 128, q0:q0 + QT], ob)

    nc.finalize()
    return nc


def kernel(x, wq, wk, wv, wo):
    import ml_dtypes
    from concourse import bass_utils

    if os.environ.get("BASS_TRACE"):
        _install_axon_ntff_hook()

    bf = ml_dtypes.bfloat16
    x = np.asarray(x, dtype=np.float32)
    wq = np.asarray(wq, dtype=np.float32)
    wk = np.asarray(wk, dtype=np.float32)
    wv = np.asarray(wv, dtype=np.float32)
    wo = np.asarray(wo, dtype=np.float32)

    # Host prep: weight slicing + rope column permutation + tables.
    perm_l = _rope_perm_local()
    perm = np.concatenate([h * HD + perm_l for h in range(NH)])  # [D]
    scale = 1.0 / np.sqrt(HD)
    wq_p = np.ascontiguousarray(wq[:, perm] * scale)
    wk_p = np.ascontiguousarray(wk[:, perm])
    cos_dup, sin_signed = _rope_tables()
    cos_dup = cos_dup.astype(bf)
    sin_signed = sin_signed.astype(bf)
    kl = np.arange(KC)[:, None]
    ql = np.arange(KC)[None, :]
    tri = np.where(ql >= kl, 0.0, MASKVAL).astype(np.float32)

    # sel[p_src, g*128 + p_dst] = 1 iff p_src == 32 * (2g + p_dst//64):
    # broadcast head (2g + p_dst//64)'s recip row onto all its 64 dims.
    sel = np.zeros((128, GQ, 128), dtype=np.float32)
    for g in range(GQ):
        for a in range(2):
            sel[32 * (2 * g + a), g, a * HD:(a + 1) * HD] = 1.0
    sel = np.ascontiguousarray(sel.reshape(128, GQ * 128).astype(bf))

    xTs = [np.ascontiguousarray(x[b].T.astype(bf)) for b in range(B)]

    in_maps = []
    for i in range(NCORES):
        b, g = divmod(i, HPC)
        cs = slice(g * DC, (g + 1) * DC)
        in_maps.append({
            "xT": xTs[b],
            "wq": np.ascontiguousarray(wq_p[:, cs].astype(bf)),
            "wk": np.ascontiguousarray(wk_p[:, cs].astype(bf)),
            "wv": np.ascontiguousarray(wv[:, cs].astype(bf)),
            "wo": np.ascontiguousarray(wo[cs, :].astype(bf)),
            "cosd": cos_dup,
            "sind": sin_signed,
            "tri": tri,
            "sel": sel,
            "vone": np.ones((128, (S // KC) * HPC), dtype=bf),
        })

    if "nc" not in _CACHE:
        _CACHE["nc"] = _build_program()
    nc = _CACHE["nc"]

    res = bass_utils.run_bass_kernel_spmd(nc, in_maps, core_ids=list(range(NCORES)))
    _CACHE["last_exec_time_ns"] = res.exec_time_ns
    _CACHE["last_res"] = res

    out = np.empty((B, S, D), dtype=np.float32)
    for b in range(B):
        acc = res.results[b * HPC]["outT"].astype(np.float32)
        for g in range(1, HPC):
            acc += res.results[b * HPC + g]["outT"].astype(np.float32)
        out[b] = acc.T
    return out


# revision 59
# speedup vs baseline: 1.2260x; 1.0279x over previous
"""Trainium2 Bass kernel for causal multi-head attention with interleaved RoPE.

Problem: B=2, S=2048, D=1024, 16 heads x 64 dims, causal, rope theta=1e4.

Sharding (8 cores): 2-way batch x 4-way head tensor-parallel.
  core i: batch b = i // 4, head group g = i % 4 (heads 4g..4g+3, dims 256).
  Each core computes q/k/v for its heads from x[b], runs causal flash
  attention, and produces a partial output projection outT [D, S].  Host
  sums the 4 partials per batch and transposes.

Performance design (v2):
  - Inputs in bf16 (halves HBM traffic); x streamed in 512-column chunks so
    projections start ~5us in instead of waiting 46us for the full load.
  - Scores: the two heads of a 128-partition group are computed as a
    row-tiled matmul pair (tile_position (0,0)/(64,0)) so both K=64
    contractions run concurrently in the PE array.
  - Both heads' score chunks live in one [128, 2, 512] PSUM tile (2 banks)
    and are EXPed by a single scalar activation -> fewer scalar instructions
    (scalar engine is the bottleneck of the attention phase; concurrent
    scalar activity also throttles the PE to ~1.2GHz, so PE work per chunk
    is halved via pairing).
  - AV: col-tiled pair (tile_position (0,0)/(0,64)) into two separate PSUM
    banks (separate banks because a matmul with start=True clears the
    has_written bits of its whole bank).
  - Softmax denominators: probs are accumulated on the Vector engine into
    sacc, reduced with M=1 ones-matmuls, inverted with one
    reciprocal_approx_fast, and broadcast back with a selection matmul --
    no DRAM round trip.
  - Output projection per q-tile right after normalize, overlapping DMA out.
"""

import os
import sys

sys.path.insert(0, "/opt/trn_rl_repo")

import numpy as np

B = 2
S = 2048
D = 1024
NH = 16
HD = 64
THETA = 10000.0
NCORES = 8
HPC = 4  # heads per core
DC = HPC * HD  # 256 dims per core
GQ = 2  # 128-partition head groups per core
QT = 512  # query tile (free dim)
NQT = S // QT
KC = 128  # key chunk (partition dim)
MASKVAL = -60.0

_CACHE = {}


def _install_axon_ntff_hook():
    """Register antenv.axon_hooks so trace=True (BASS_TRACE=1) works."""
    import types

    if "antenv.axon_hooks" in sys.modules:
        return
    m = types.ModuleType("antenv.axon_hooks")
    _hook = [None]
    m.set_axon_ntff_profile_hook = lambda h: _hook.__setitem__(0, h)
    m.get_axon_ntff_profile_hook = lambda: _hook[0]
    sys.modules["antenv.axon_hooks"] = m
    try:
        import antenv

        antenv.axon_hooks = m
        from trn_agent_boot.trn_boot import _ntff_profile_via_ctypes

        hook = _ntff_profile_via_ctypes("/opt/axon/libaxon_pjrt.so")
        if hook is not None:
            m.set_axon_ntff_profile_hook(hook)
    except Exception:
        pass


def _rope_perm_local():
    """Permutation of one head's 64 dims: original interleaved pair (2i, 2i+1)
    -> t0 at quadrant*32 + (i%16), t1 at quadrant*32 + 16 + (i%16), with
    quadrant = i // 16.  Returns perm such that new[j] = old[perm[j]]."""
    perm = np.zeros(HD, dtype=np.int64)
    for i in range(HD // 2):
        qd, r = divmod(i, 16)
        perm[qd * 32 + r] = 2 * i
        perm[qd * 32 + 16 + r] = 2 * i + 1
    return perm


def _rope_tables():
    """cos_dup/sin_signed [128, S]: per-partition rope tables matching the
    de-interleaved layout (pattern repeats every 64 partitions)."""
    inv_freq = 1.0 / (THETA ** (np.arange(0, HD, 2, dtype=np.float64) / HD))  # [32]
    pos = np.arange(S, dtype=np.float64)
    ang = pos[None, :] * inv_freq[:, None]  # [32, S]
    cos = np.cos(ang)
    sin = np.sin(ang)
    cos_dup = np.zeros((128, S), dtype=np.float32)
    sin_signed = np.zeros((128, S), dtype=np.float32)
    for p in range(128):
        d = p % HD
        qd, r0 = divmod(d, 32)
        if r0 < 16:
            i = qd * 16 + r0
            cos_dup[p] = cos[i]
            sin_signed[p] = -sin[i]
        else:
            i = qd * 16 + (r0 - 16)
            cos_dup[p] = cos[i]
            sin_signed[p] = sin[i]
    return cos_dup, sin_signed


def _build_program():
    import concourse.bass as bass
    from concourse import bacc, mybir
    import concourse.tile as tile

    f32 = mybir.dt.float32
    f32r = mybir.dt.float32r
    bf16 = mybir.dt.bfloat16
    ADD = mybir.AluOpType.add
    MULT = mybir.AluOpType.mult
    EXP = mybir.ActivationFunctionType.Exp
    SWAP16 = [(j + 16) % 32 for j in range(32)]
    DK = D // 128  # 8 contraction chunks

    nc = bacc.Bacc("TRN2", target_bir_lowering=False, debug=False)
    xT = nc.dram_tensor("xT", [D, S], bf16, kind="ExternalInput").ap()
    wq = nc.dram_tensor("wq", [D, DC], bf16, kind="ExternalInput").ap()
    wk = nc.dram_tensor("wk", [D, DC], bf16, kind="ExternalInput").ap()
    wv = nc.dram_tensor("wv", [D, DC], bf16, kind="ExternalInput").ap()
    wo = nc.dram_tensor("wo", [DC, D], bf16, kind="ExternalInput").ap()
    cosd = nc.dram_tensor("cosd", [128, S], bf16, kind="ExternalInput").ap()
    sind = nc.dram_tensor("sind", [128, S], bf16, kind="ExternalInput").ap()
    tri = nc.dram_tensor("tri", [KC, KC], f32, kind="ExternalInput").ap()
    sel = nc.dram_tensor("sel", [128, GQ * 128], bf16,
                         kind="ExternalInput").ap()
    vone = nc.dram_tensor("vone", [128, (S // KC) * HPC], bf16,
                          kind="ExternalInput").ap()
    outT = nc.dram_tensor("outT", [D, S], bf16, kind="ExternalOutput").ap()

    with tile.TileContext(nc) as tc:
        with tc.tile_pool(name="const", bufs=1) as const:
            cos_sb = const.tile([128, S], bf16)
            sin_sb = const.tile([128, S], bf16)
            tri_sb = const.tile([KC, KC], f32)
            wq_sb = const.tile([128, DK, DC], bf16)
            wk_sb = const.tile([128, DK, DC], bf16)
            wv_sb = const.tile([128, DK, DC], bf16)
            wo_sb = const.tile([128, GQ, D], bf16)
            xT_sb = const.tile([128, DK, S], bf16)
            qT_sb = const.tile([128, GQ, S], bf16)
            kT_sb = const.tile([128, GQ, S], bf16)
            vaug_sb = const.tile([128, S // KC, HPC * (HD + 1)], bf16)
            oT_sb = const.tile([128, GQ, S], bf16)
            sums_sb = const.tile([128, S], f32)
            recip_sb = const.tile([128, S], f32)
            recip_bf = const.tile([128, S], bf16)
            sel_sb = const.tile([128, GQ, 128], bf16)

            # DMA order tuned so phase 1 can start ~5us in.
            xTr = xT.rearrange("(o p) n -> p o n", p=128)
            nc.sync.dma_start(wq_sb, wq.rearrange("(o p) n -> p o n", p=128))
            nc.sync.dma_start(xT_sb[:, :, 0:QT], xTr[:, :, 0:QT])
            nc.sync.dma_start(wk_sb, wk.rearrange("(o p) n -> p o n", p=128))
            nc.sync.dma_start(cos_sb[:, 0:QT], cosd[:, 0:QT])
            nc.sync.dma_start(sin_sb[:, 0:QT], sind[:, 0:QT])
            nc.sync.dma_start(wv_sb, wv.rearrange("(o p) n -> p o n", p=128))
            nc.sync.dma_start(tri_sb, tri)
            nc.sync.dma_start(
                vaug_sb[:, :, HD::(HD + 1)],
                vone.rearrange("p (a b) -> p a b", a=S // KC))
            nc.sync.dma_start(
                sel_sb, sel.rearrange("p (c n) -> p c n", c=GQ))
            for qt in range(1, NQT):
                q0 = qt * QT
                nc.sync.dma_start(xT_sb[:, :, q0:q0 + QT], xTr[:, :, q0:q0 + QT])
                nc.sync.dma_start(cos_sb[:, q0:q0 + QT], cosd[:, q0:q0 + QT])
                nc.sync.dma_start(sin_sb[:, q0:q0 + QT], sind[:, q0:q0 + QT])
            nc.sync.dma_start(wo_sb, wo.rearrange("(o p) n -> p o n", p=128))

            # ---- Phase 1: q/k/v projections + rope (PE warm, scalar idle) ----
            with tc.tile_pool(name="p1", bufs=2, space="PSUM") as p1, \
                 tc.tile_pool(name="tmp1", bufs=3) as tmp1:
                def rope(ps, dst, q0):
                    # pure-bf16 chain after one psum cast: 2x DVE throughput
                    qb = tmp1.tile([128, QT], bf16, tag="qb")
                    nc.vector.tensor_copy(out=qb, in_=ps)
                    shuf = tmp1.tile([128, QT], bf16, tag="shuf")
                    nc.vector.stream_shuffle(shuf, qb, SWAP16)
                    m1 = tmp1.tile([128, QT], bf16, tag="m1")
                    nc.vector.tensor_tensor(m1, qb, cos_sb[:, q0:q0 + QT], MULT)
                    m2 = tmp1.tile([128, QT], bf16, tag="m2")
                    nc.vector.tensor_tensor(m2, shuf, sin_sb[:, q0:q0 + QT], MULT)
                    nc.vector.tensor_tensor(dst, m1, m2, ADD)

                import concourse.bass as _b
                for qt in range(NQT):
                    q0 = qt * QT
                    for g in range(GQ):
                        ps_q = p1.tile([128, QT], f32, tag="q")
                        for kc in range(DK):
                            nc.tensor.matmul(
                                ps_q, wq_sb[:, kc, g * 128:(g + 1) * 128],
                                xT_sb[:, kc, q0:q0 + QT],
                                start=(kc == 0), stop=(kc == DK - 1))
                        ps_k = p1.tile([128, QT], f32, tag="k")
                        for kc in range(DK):
                            nc.tensor.matmul(
                                ps_k, wk_sb[:, kc, g * 128:(g + 1) * 128],
                                xT_sb[:, kc, q0:q0 + QT],
                                start=(kc == 0), stop=(kc == DK - 1))
                        rope(ps_q, qT_sb[:, g, q0:q0 + QT], q0)
                        rope(ps_k, kT_sb[:, g, q0:q0 + QT], q0)
                    for rc in range(QT // KC):
                        r0 = q0 + rc * KC
                        ps_v = p1.tile([128, DC], f32, tag="v", bufs=4)
                        for kc in range(DK):
                            nc.tensor.matmul(
                                ps_v, xT_sb[:, kc, r0:r0 + KC],
                                wv_sb[:, kc, :],
                                start=(kc == 0), stop=(kc == DK - 1))
                        # one strided copy: psum [128,(h d)] -> vaug 65-pitch
                        vdst = vaug_sb[:, r0 // KC, 0:HD]
                        dst3 = _b.AP(tensor=vdst.tensor, offset=vdst.offset,
                                     ap=[list(vdst.ap[0]), [HD + 1, HPC],
                                         [1, HD]])
                        src3 = _b.AP(tensor=ps_v.tensor, offset=ps_v.offset,
                                     ap=[list(ps_v.ap[0]), [HD, HPC],
                                         [1, HD]])
                        nc.vector.tensor_copy(out=dst3, in_=src3)

            # ---- Phase 3: causal flash attention (S^T orientation) ----
            # Scores for the two heads of a group are a row-tiled concurrent
            # pair into one 2-bank PSUM tile, EXPed by a single scalar op.
            # AV uses the ones-row trick (M=65): psum row 64 = softmax sums.
            with tc.tile_pool(name="pss", bufs=2, space="PSUM") as pss, \
                 tc.tile_pool(name="po", bufs=2, space="PSUM") as po, \
                 tc.tile_pool(name="ppr", bufs=5) as ppr:
                # Software-pipelined: AV of chunk i is emitted after the
                # S-pair of chunk i+2, so the PE never queues behind EXP(i)
                # even when the diag-chunk tri-add delays EXP.
                chunks = []
                for g in range(GQ):
                    for qt in range(NQT):
                        nkc = (qt * QT + QT) // KC
                        for kc in range(nkc):
                            chunks.append((g, qt, kc, nkc))

                # sums_sb init to 1.0 (memset >1 column miscompiles): garbage
                # lanes must stay finite-nonzero for the batched reciprocal.
                # Done here: phase-1's vector engine is saturated, attention's
                # has slack, and the first sums-row write lands much later.
                for qt in range(NQT):
                    q0 = qt * QT
                    nc.vector.tensor_scalar(
                        sums_sb[:, q0:q0 + QT], cos_sb[:, q0:q0 + QT],
                        0.0, 1.0, MULT, ADD)

                state = {}  # (g, qt) -> ps_o pair
                pendq = []  # [(g, qt, kc, nkc, probs, qlo)]

                def emit_av(p):
                    g, qt, kc, nkc, probs, qlo = p
                    q0 = qt * QT
                    for a in range(2):
                        h = 2 * g + a
                        nc.tensor.matmul(
                            state[(g, qt)][a][:, qlo:QT],
                            vaug_sb[:, kc, h * (HD + 1):(h + 1) * (HD + 1)],
                            probs[:, a, qlo:QT],
                            start=(kc == 0), stop=(kc == nkc - 1))
                    if kc == nkc - 1:
                        for a in range(2):
                            h = 2 * g + a
                            nc.vector.tensor_copy(
                                out=oT_sb[a * HD:(a + 1) * HD, g, q0:q0 + QT],
                                in_=state[(g, qt)][a][0:HD, :])
                            nc.vector.tensor_copy(
                                out=sums_sb[32 * h:32 * h + 1, q0:q0 + QT],
                                in_=state[(g, qt)][a][HD:HD + 1, :])
                        del state[(g, qt)]
                        if (g, qt) == (GQ - 1, NQT - 2):
                            # all sums except qt=NQT-1 are final: invert the
                            # first 3/4 while the last q-tile is still running
                            nc.vector.reciprocal_approx_fast(
                                recip_sb[:, 0:(NQT - 1) * QT],
                                sums_sb[:, 0:(NQT - 1) * QT])
                            nc.vector.tensor_copy(
                                out=recip_bf[:, 0:(NQT - 1) * QT],
                                in_=recip_sb[:, 0:(NQT - 1) * QT])

                for g, qt, kc, nkc in chunks:
                    q0 = qt * QT
                    k0 = kc * KC
                    j = k0 - q0
                    qlo = max(0, j)
                    if kc == 0:
                        state[(g, qt)] = [
                            po.tile([HD + 1, QT], f32, tag=f"o{a}",
                                    name=f"ps_o{g}_{qt}_{a}")
                            for a in range(2)]
                    ps_s = pss.tile([128, 2, QT], f32, tag="s")
                    for a in range(2):
                        nc.tensor.matmul(
                            ps_s[:, a, qlo:QT],
                            kT_sb[a * HD:(a + 1) * HD, g, k0:k0 + KC],
                            qT_sb[a * HD:(a + 1) * HD, g, q0 + qlo:q0 + QT],
                            start=True, stop=True)
                    if len(pendq) >= 2:
                        emit_av(pendq.pop(0))
                    if j >= 0:
                        for a in range(2):
                            nc.vector.tensor_tensor(
                                ps_s[:, a, qlo:qlo + KC],
                                ps_s[:, a, qlo:qlo + KC], tri_sb, ADD)
                    probs = ppr.tile([128, 2, QT], bf16, tag="p")
                    nc.scalar.activation(
                        probs[:, :, qlo:QT], ps_s[:, :, qlo:QT], EXP)
                    pendq.append((g, qt, kc, nkc, probs, qlo))
                for p in pendq:
                    emit_av(p)

            # ---- Phase 4+5: normalize and output projection ----
            with tc.tile_pool(name="p5", bufs=4, space="PSUM") as p5, \
                 tc.tile_pool(name="pr", bufs=2, space="PSUM") as pr, \
                 tc.tile_pool(name="p5s", bufs=6) as p5s:
                # last q-tile's reciprocal (first 3/4 were emitted inside the
                # attention loop to hide the latency)
                q3 = (NQT - 1) * QT
                nc.vector.reciprocal_approx_fast(
                    recip_sb[:, q3:S], sums_sb[:, q3:S])
                nc.vector.tensor_copy(
                    out=recip_bf[:, q3:S], in_=recip_sb[:, q3:S])
                for qt in range(NQT):
                    q0 = qt * QT
                    for g in range(GQ):
                        ps_r = pr.tile([128, QT], f32, tag="r")
                        nc.tensor.matmul(ps_r, sel_sb[:, g, :],
                                         recip_bf[:, q0:q0 + QT],
                                         start=True, stop=True)
                        nc.vector.tensor_tensor(
                            oT_sb[:, g, q0:q0 + QT], oT_sb[:, g, q0:q0 + QT],
                            ps_r, MULT)
                for qt in range(NQT):
                    q0 = qt * QT
                    for ec in range(D // 128):
                        ps = p5.tile([128, QT], f32, tag="f")
                        for g in range(GQ):
                            nc.tensor.matmul(
                                ps, wo_sb[:, g, ec * 128:(ec + 1) * 128],
                                oT_sb[:, g, q0:q0 + QT],
                                start=(g == 0), stop=(g == GQ - 1))
                        ob = p5s.tile([128, QT], bf16, tag="ob")
                        nc.scalar.copy(out=ob[:, 0:QT // 2], in_=ps[:, 0:QT // 2])
                        nc.vector.tensor_copy(out=ob[:, QT // 2:QT],
                                              in_=ps[:, QT // 2:QT])
                        nc.sync.dma_start(
                            outT[ec * 128:(ec + 1) * 128, q0:q0 + QT], ob)

    nc.finalize()
    return nc


def kernel(x, wq, wk, wv, wo):
    import ml_dtypes
    from concourse import bass_utils

    if os.environ.get("BASS_TRACE"):
        _install_axon_ntff_hook()

    bf = ml_dtypes.bfloat16
    x = np.asarray(x, dtype=np.float32)
    wq = np.asarray(wq, dtype=np.float32)
    wk = np.asarray(wk, dtype=np.float32)
    wv = np.asarray(wv, dtype=np.float32)
    wo = np.asarray(wo, dtype=np.float32)

    # Host prep: weight slicing + rope column permutation + tables.
    perm_l = _rope_perm_local()
    perm = np.concatenate([h * HD + perm_l for h in range(NH)])  # [D]
    scale = 1.0 / np.sqrt(HD)
    wq_p = np.ascontiguousarray(wq[:, perm] * scale)
    wk_p = np.ascontiguousarray(wk[:, perm])
    cos_dup, sin_signed = _rope_tables()
    cos_dup = cos_dup.astype(bf)
    sin_signed = sin_signed.astype(bf)
    kl = np.arange(KC)[:, None]
    ql = np.arange(KC)[None, :]
    tri = np.where(ql >= kl, 0.0, MASKVAL).astype(np.float32)

    # sel[p_src, g*128 + p_dst] = 1 iff p_src == 32 * (2g + p_dst//64):
    # broadcast head (2g + p_dst//64)'s recip row onto all its 64 dims.
    sel = np.zeros((128, GQ, 128), dtype=np.float32)
    for g in range(GQ):
        for a in range(2):
            sel[32 * (2 * g + a), g, a * HD:(a + 1) * HD] = 1.0
    sel = np.ascontiguousarray(sel.reshape(128, GQ * 128).astype(bf))

    xTs = [np.ascontiguousarray(x[b].T.astype(bf)) for b in range(B)]

    in_maps = []
    for i in range(NCORES):
        b, g = divmod(i, HPC)
        cs = slice(g * DC, (g + 1) * DC)
        in_maps.append({
            "xT": xTs[b],
            "wq": np.ascontiguousarray(wq_p[:, cs].astype(bf)),
            "wk": np.ascontiguousarray(wk_p[:, cs].astype(bf)),
            "wv": np.ascontiguousarray(wv[:, cs].astype(bf)),
            "wo": np.ascontiguousarray(wo[cs, :].astype(bf)),
            "cosd": cos_dup,
            "sind": sin_signed,
            "tri": tri,
            "sel": sel,
            "vone": np.ones((128, (S // KC) * HPC), dtype=bf),
        })

    if "nc" not in _CACHE:
        _CACHE["nc"] = _build_program()
    nc = _CACHE["nc"]

    res = bass_utils.run_bass_kernel_spmd(nc, in_maps, core_ids=list(range(NCORES)))
    _CACHE["last_exec_time_ns"] = res.exec_time_ns
    _CACHE["last_res"] = res

    out = np.empty((B, S, D), dtype=np.float32)
    for b in range(B):
        acc = res.results[b * HPC]["outT"].astype(np.float32)
        for g in range(1, HPC):
            acc += res.results[b * HPC + g]["outT"].astype(np.float32)
        out[b] = acc.T
    return out


# revision 60
# speedup vs baseline: 1.2290x; 1.0025x over previous
"""Trainium2 Bass kernel for causal multi-head attention with interleaved RoPE.

Problem: B=2, S=2048, D=1024, 16 heads x 64 dims, causal, rope theta=1e4.

Sharding (8 cores): 2-way batch x 4-way head tensor-parallel.
  core i: batch b = i // 4, head group g = i % 4 (heads 4g..4g+3, dims 256).
  Each core computes q/k/v for its heads from x[b], runs causal flash
  attention, and produces a partial output projection outT [D, S].  Host
  sums the 4 partials per batch and transposes.

Performance design (v2):
  - Inputs in bf16 (halves HBM traffic); x streamed in 512-column chunks so
    projections start ~5us in instead of waiting 46us for the full load.
  - Scores: the two heads of a 128-partition group are computed as a
    row-tiled matmul pair (tile_position (0,0)/(64,0)) so both K=64
    contractions run concurrently in the PE array.
  - Both heads' score chunks live in one [128, 2, 512] PSUM tile (2 banks)
    and are EXPed by a single scalar activation -> fewer scalar instructions
    (scalar engine is the bottleneck of the attention phase; concurrent
    scalar activity also throttles the PE to ~1.2GHz, so PE work per chunk
    is halved via pairing).
  - AV: col-tiled pair (tile_position (0,0)/(0,64)) into two separate PSUM
    banks (separate banks because a matmul with start=True clears the
    has_written bits of its whole bank).
  - Softmax denominators: probs are accumulated on the Vector engine into
    sacc, reduced with M=1 ones-matmuls, inverted with one
    reciprocal_approx_fast, and broadcast back with a selection matmul --
    no DRAM round trip.
  - Output projection per q-tile right after normalize, overlapping DMA out.
"""

import os
import sys

sys.path.insert(0, "/opt/trn_rl_repo")

import numpy as np

B = 2
S = 2048
D = 1024
NH = 16
HD = 64
THETA = 10000.0
NCORES = 8
HPC = 4  # heads per core
DC = HPC * HD  # 256 dims per core
GQ = 2  # 128-partition head groups per core
QT = 512  # query tile (free dim)
NQT = S // QT
KC = 128  # key chunk (partition dim)
MASKVAL = -60.0

_CACHE = {}


def _install_axon_ntff_hook():
    """Register antenv.axon_hooks so trace=True (BASS_TRACE=1) works."""
    import types

    if "antenv.axon_hooks" in sys.modules:
        return
    m = types.ModuleType("antenv.axon_hooks")
    _hook = [None]
    m.set_axon_ntff_profile_hook = lambda h: _hook.__setitem__(0, h)
    m.get_axon_ntff_profile_hook = lambda: _hook[0]
    sys.modules["antenv.axon_hooks"] = m
    try:
        import antenv

        antenv.axon_hooks = m
        from trn_agent_boot.trn_boot import _ntff_profile_via_ctypes

        hook = _ntff_profile_via_ctypes("/opt/axon/libaxon_pjrt.so")
        if hook is not None:
            m.set_axon_ntff_profile_hook(hook)
    except Exception:
        pass


def _rope_perm_local():
    """Permutation of one head's 64 dims: original interleaved pair (2i, 2i+1)
    -> t0 at quadrant*32 + (i%16), t1 at quadrant*32 + 16 + (i%16), with
    quadrant = i // 16.  Returns perm such that new[j] = old[perm[j]]."""
    perm = np.zeros(HD, dtype=np.int64)
    for i in range(HD // 2):
        qd, r = divmod(i, 16)
        perm[qd * 32 + r] = 2 * i
        perm[qd * 32 + 16 + r] = 2 * i + 1
    return perm


def _rope_tables():
    """cos_dup/sin_signed [128, S]: per-partition rope tables matching the
    de-interleaved layout (pattern repeats every 64 partitions)."""
    inv_freq = 1.0 / (THETA ** (np.arange(0, HD, 2, dtype=np.float64) / HD))  # [32]
    pos = np.arange(S, dtype=np.float64)
    ang = pos[None, :] * inv_freq[:, None]  # [32, S]
    cos = np.cos(ang)
    sin = np.sin(ang)
    cos_dup = np.zeros((128, S), dtype=np.float32)
    sin_signed = np.zeros((128, S), dtype=np.float32)
    for p in range(128):
        d = p % HD
        qd, r0 = divmod(d, 32)
        if r0 < 16:
            i = qd * 16 + r0
            cos_dup[p] = cos[i]
            sin_signed[p] = -sin[i]
        else:
            i = qd * 16 + (r0 - 16)
            cos_dup[p] = cos[i]
            sin_signed[p] = sin[i]
    return cos_dup, sin_signed


def _build_program():
    import concourse.bass as bass
    from concourse import bacc, mybir
    import concourse.tile as tile

    f32 = mybir.dt.float32
    f32r = mybir.dt.float32r
    bf16 = mybir.dt.bfloat16
    ADD = mybir.AluOpType.add
    MULT = mybir.AluOpType.mult
    EXP = mybir.ActivationFunctionType.Exp
    SWAP16 = [(j + 16) % 32 for j in range(32)]
    DK = D // 128  # 8 contraction chunks

    nc = bacc.Bacc("TRN2", target_bir_lowering=False, debug=False)
    xT = nc.dram_tensor("xT", [D, S], bf16, kind="ExternalInput").ap()
    wq = nc.dram_tensor("wq", [D, DC], bf16, kind="ExternalInput").ap()
    wk = nc.dram_tensor("wk", [D, DC], bf16, kind="ExternalInput").ap()
    wv = nc.dram_tensor("wv", [D, DC], bf16, kind="ExternalInput").ap()
    wo = nc.dram_tensor("wo", [DC, D], bf16, kind="ExternalInput").ap()
    cosd = nc.dram_tensor("cosd", [128, S], bf16, kind="ExternalInput").ap()
    sind = nc.dram_tensor("sind", [128, S], bf16, kind="ExternalInput").ap()
    tri = nc.dram_tensor("tri", [KC, KC], f32, kind="ExternalInput").ap()
    sel = nc.dram_tensor("sel", [128, GQ * 128], bf16,
                         kind="ExternalInput").ap()
    vone = nc.dram_tensor("vone", [128, (S // KC) * HPC], bf16,
                          kind="ExternalInput").ap()
    outT = nc.dram_tensor("outT", [D, S], bf16, kind="ExternalOutput").ap()

    with tile.TileContext(nc) as tc:
        with tc.tile_pool(name="const", bufs=1) as const:
            cos_sb = const.tile([128, S], bf16)
            sin_sb = const.tile([128, S], bf16)
            tri_sb = const.tile([KC, KC], f32)
            wq_sb = const.tile([128, DK, DC], bf16)
            wk_sb = const.tile([128, DK, DC], bf16)
            wv_sb = const.tile([128, DK, DC], bf16)
            wo_sb = const.tile([128, GQ, D], bf16)
            xT_sb = const.tile([128, DK, S], bf16)
            qT_sb = const.tile([128, GQ, S], bf16)
            kT_sb = const.tile([128, GQ, S], bf16)
            vaug_sb = const.tile([128, S // KC, HPC * (HD + 1)], bf16)
            oT_sb = const.tile([128, GQ, S], bf16)
            sums_sb = const.tile([128, S], f32)
            recip_sb = const.tile([128, S], f32)
            recip_bf = const.tile([128, S], bf16)
            sel_sb = const.tile([128, GQ, 128], bf16)

            # DMA order tuned so phase 1 can start ~5us in.
            xTr = xT.rearrange("(o p) n -> p o n", p=128)
            nc.sync.dma_start(wq_sb, wq.rearrange("(o p) n -> p o n", p=128))
            nc.sync.dma_start(xT_sb[:, :, 0:QT], xTr[:, :, 0:QT])
            nc.sync.dma_start(wk_sb, wk.rearrange("(o p) n -> p o n", p=128))
            nc.sync.dma_start(cos_sb[:, 0:QT], cosd[:, 0:QT])
            nc.sync.dma_start(sin_sb[:, 0:QT], sind[:, 0:QT])
            nc.sync.dma_start(wv_sb, wv.rearrange("(o p) n -> p o n", p=128))
            nc.sync.dma_start(tri_sb, tri)
            nc.sync.dma_start(
                vaug_sb[:, :, HD::(HD + 1)],
                vone.rearrange("p (a b) -> p a b", a=S // KC))
            nc.sync.dma_start(
                sel_sb, sel.rearrange("p (c n) -> p c n", c=GQ))
            for qt in range(1, NQT):
                q0 = qt * QT
                nc.sync.dma_start(xT_sb[:, :, q0:q0 + QT], xTr[:, :, q0:q0 + QT])
                nc.sync.dma_start(cos_sb[:, q0:q0 + QT], cosd[:, q0:q0 + QT])
                nc.sync.dma_start(sin_sb[:, q0:q0 + QT], sind[:, q0:q0 + QT])
            nc.sync.dma_start(wo_sb, wo.rearrange("(o p) n -> p o n", p=128))

            # ---- Phase 1: q/k/v projections + rope (PE warm, scalar idle) ----
            with tc.tile_pool(name="p1", bufs=2, space="PSUM") as p1, \
                 tc.tile_pool(name="tmp1", bufs=3) as tmp1:
                def rope(ps, dst, q0):
                    # pure-bf16 chain after one psum cast: 2x DVE throughput
                    qb = tmp1.tile([128, QT], bf16, tag="qb")
                    nc.vector.tensor_copy(out=qb, in_=ps)
                    shuf = tmp1.tile([128, QT], bf16, tag="shuf")
                    nc.vector.stream_shuffle(shuf, qb, SWAP16)
                    m1 = tmp1.tile([128, QT], bf16, tag="m1")
                    nc.vector.tensor_tensor(m1, qb, cos_sb[:, q0:q0 + QT], MULT)
                    m2 = tmp1.tile([128, QT], bf16, tag="m2")
                    nc.vector.tensor_tensor(m2, shuf, sin_sb[:, q0:q0 + QT], MULT)
                    nc.vector.tensor_tensor(dst, m1, m2, ADD)

                import concourse.bass as _b
                for qt in range(NQT):
                    q0 = qt * QT
                    for g in range(GQ):
                        ps_q = p1.tile([128, QT], f32, tag="q")
                        for kc in range(DK):
                            nc.tensor.matmul(
                                ps_q, wq_sb[:, kc, g * 128:(g + 1) * 128],
                                xT_sb[:, kc, q0:q0 + QT],
                                start=(kc == 0), stop=(kc == DK - 1))
                        ps_k = p1.tile([128, QT], f32, tag="k")
                        for kc in range(DK):
                            nc.tensor.matmul(
                                ps_k, wk_sb[:, kc, g * 128:(g + 1) * 128],
                                xT_sb[:, kc, q0:q0 + QT],
                                start=(kc == 0), stop=(kc == DK - 1))
                        rope(ps_q, qT_sb[:, g, q0:q0 + QT], q0)
                        rope(ps_k, kT_sb[:, g, q0:q0 + QT], q0)
                    for rc in range(QT // KC):
                        r0 = q0 + rc * KC
                        ps_v = p1.tile([128, DC], f32, tag="v", bufs=4)
                        for kc in range(DK):
                            nc.tensor.matmul(
                                ps_v, xT_sb[:, kc, r0:r0 + KC],
                                wv_sb[:, kc, :],
                                start=(kc == 0), stop=(kc == DK - 1))
                        # one strided copy: psum [128,(h d)] -> vaug 65-pitch
                        vdst = vaug_sb[:, r0 // KC, 0:HD]
                        dst3 = _b.AP(tensor=vdst.tensor, offset=vdst.offset,
                                     ap=[list(vdst.ap[0]), [HD + 1, HPC],
                                         [1, HD]])
                        src3 = _b.AP(tensor=ps_v.tensor, offset=ps_v.offset,
                                     ap=[list(ps_v.ap[0]), [HD, HPC],
                                         [1, HD]])
                        nc.vector.tensor_copy(out=dst3, in_=src3)

            # ---- Phase 3: causal flash attention (S^T orientation) ----
            # Scores for the two heads of a group are a row-tiled concurrent
            # pair into one 2-bank PSUM tile, EXPed by a single scalar op.
            # AV uses the ones-row trick (M=65): psum row 64 = softmax sums.
            with tc.tile_pool(name="pss", bufs=2, space="PSUM") as pss, \
                 tc.tile_pool(name="po", bufs=2, space="PSUM") as po, \
                 tc.tile_pool(name="ppr", bufs=5) as ppr:
                # Software-pipelined: AV of chunk i is emitted after the
                # S-pair of chunk i+2, so the PE never queues behind EXP(i)
                # even when the diag-chunk tri-add delays EXP.
                chunks = []
                for g in range(GQ):
                    for qt in range(NQT):
                        nkc = (qt * QT + QT) // KC
                        for kc in range(nkc):
                            chunks.append((g, qt, kc, nkc))

                # sums_sb init to 1.0 (memset >1 column miscompiles): garbage
                # lanes must stay finite-nonzero for the batched reciprocal.
                # Done here: phase-1's vector engine is saturated, attention's
                # has slack, and the first sums-row write lands much later.
                for qt in range(NQT):
                    q0 = qt * QT
                    nc.vector.tensor_scalar(
                        sums_sb[:, q0:q0 + QT], cos_sb[:, q0:q0 + QT],
                        0.0, 1.0, MULT, ADD)

                state = {}  # (g, qt) -> ps_o pair
                pendq = []  # [(g, qt, kc, nkc, probs, qlo)]

                def emit_av(p):
                    g, qt, kc, nkc, probs, qlo = p
                    q0 = qt * QT
                    for a in range(2):
                        h = 2 * g + a
                        nc.tensor.matmul(
                            state[(g, qt)][a][:, qlo:QT],
                            vaug_sb[:, kc, h * (HD + 1):(h + 1) * (HD + 1)],
                            probs[:, a, qlo:QT],
                            start=(kc == 0), stop=(kc == nkc - 1))
                    if kc == nkc - 1:
                        for a in range(2):
                            h = 2 * g + a
                            nc.vector.tensor_copy(
                                out=oT_sb[a * HD:(a + 1) * HD, g, q0:q0 + QT],
                                in_=state[(g, qt)][a][0:HD, :])
                            nc.vector.tensor_copy(
                                out=sums_sb[32 * h:32 * h + 1, q0:q0 + QT],
                                in_=state[(g, qt)][a][HD:HD + 1, :])
                        del state[(g, qt)]
                        if (g, qt) == (GQ - 1, NQT - 2):
                            # all sums except qt=NQT-1 are final: invert the
                            # first 3/4 while the last q-tile is still running
                            nc.vector.reciprocal_approx_fast(
                                recip_sb[:, 0:(NQT - 1) * QT],
                                sums_sb[:, 0:(NQT - 1) * QT])
                            nc.vector.tensor_copy(
                                out=recip_bf[:, 0:(NQT - 1) * QT],
                                in_=recip_sb[:, 0:(NQT - 1) * QT])

                for g, qt, kc, nkc in chunks:
                    q0 = qt * QT
                    k0 = kc * KC
                    j = k0 - q0
                    qlo = max(0, j)
                    if kc == 0:
                        state[(g, qt)] = [
                            po.tile([HD + 1, QT], f32, tag=f"o{a}",
                                    name=f"ps_o{g}_{qt}_{a}")
                            for a in range(2)]
                    ps_s = pss.tile([128, 2, QT], f32, tag="s")
                    for a in range(2):
                        nc.tensor.matmul(
                            ps_s[:, a, qlo:QT],
                            kT_sb[a * HD:(a + 1) * HD, g, k0:k0 + KC],
                            qT_sb[a * HD:(a + 1) * HD, g, q0 + qlo:q0 + QT],
                            start=True, stop=True)
                    if len(pendq) >= 2:
                        emit_av(pendq.pop(0))
                    if j >= 0:
                        for a in range(2):
                            nc.vector.tensor_tensor(
                                ps_s[:, a, qlo:qlo + KC],
                                ps_s[:, a, qlo:qlo + KC], tri_sb, ADD)
                    probs = ppr.tile([128, 2, QT], bf16, tag="p")
                    nc.scalar.activation(
                        probs[:, :, qlo:QT], ps_s[:, :, qlo:QT], EXP)
                    pendq.append((g, qt, kc, nkc, probs, qlo))
                for p in pendq:
                    emit_av(p)

            # ---- Phase 4+5: normalize and output projection ----
            with tc.tile_pool(name="p5", bufs=4, space="PSUM") as p5, \
                 tc.tile_pool(name="pr", bufs=2, space="PSUM") as pr, \
                 tc.tile_pool(name="p5s", bufs=6) as p5s:
                # last q-tile's reciprocal (first 3/4 were emitted inside the
                # attention loop to hide the latency)
                q3 = (NQT - 1) * QT
                nc.vector.reciprocal_approx_fast(
                    recip_sb[:, q3:S], sums_sb[:, q3:S])
                nc.vector.tensor_copy(
                    out=recip_bf[:, q3:S], in_=recip_sb[:, q3:S])
                for qt in range(NQT):
                    q0 = qt * QT
                    for g in range(GQ):
                        ps_r = pr.tile([128, QT], f32, tag="r")
                        nc.tensor.matmul(ps_r, sel_sb[:, g, :],
                                         recip_bf[:, q0:q0 + QT],
                                         start=True, stop=True)
                        nc.vector.tensor_tensor(
                            oT_sb[:, g, q0:q0 + QT], oT_sb[:, g, q0:q0 + QT],
                            ps_r, MULT)
                for qt in range(NQT):
                    q0 = qt * QT
                    for ec in range(D // 128):
                        ps = p5.tile([128, QT], f32, tag="f")
                        for g in range(GQ):
                            nc.tensor.matmul(
                                ps, wo_sb[:, g, ec * 128:(ec + 1) * 128],
                                oT_sb[:, g, q0:q0 + QT],
                                start=(g == 0), stop=(g == GQ - 1))
                        ob = p5s.tile([128, QT], bf16, tag="ob")
                        # vector only: scalar-engine activity throttles the PE
                        nc.vector.tensor_copy(out=ob, in_=ps)
                        nc.sync.dma_start(
                            outT[ec * 128:(ec + 1) * 128, q0:q0 + QT], ob)

    nc.finalize()
    return nc


def kernel(x, wq, wk, wv, wo):
    import ml_dtypes
    from concourse import bass_utils

    if os.environ.get("BASS_TRACE"):
        _install_axon_ntff_hook()

    bf = ml_dtypes.bfloat16
    x = np.asarray(x, dtype=np.float32)
    wq = np.asarray(wq, dtype=np.float32)
    wk = np.asarray(wk, dtype=np.float32)
    wv = np.asarray(wv, dtype=np.float32)
    wo = np.asarray(wo, dtype=np.float32)

    # Host prep: weight slicing + rope column permutation + tables.
    perm_l = _rope_perm_local()
    perm = np.concatenate([h * HD + perm_l for h in range(NH)])  # [D]
    scale = 1.0 / np.sqrt(HD)
    wq_p = np.ascontiguousarray(wq[:, perm] * scale)
    wk_p = np.ascontiguousarray(wk[:, perm])
    cos_dup, sin_signed = _rope_tables()
    cos_dup = cos_dup.astype(bf)
    sin_signed = sin_signed.astype(bf)
    kl = np.arange(KC)[:, None]
    ql = np.arange(KC)[None, :]
    tri = np.where(ql >= kl, 0.0, MASKVAL).astype(np.float32)

    # sel[p_src, g*128 + p_dst] = 1 iff p_src == 32 * (2g + p_dst//64):
    # broadcast head (2g + p_dst//64)'s recip row onto all its 64 dims.
    sel = np.zeros((128, GQ, 128), dtype=np.float32)
    for g in range(GQ):
        for a in range(2):
            sel[32 * (2 * g + a), g, a * HD:(a + 1) * HD] = 1.0
    sel = np.ascontiguousarray(sel.reshape(128, GQ * 128).astype(bf))

    xTs = [np.ascontiguousarray(x[b].T.astype(bf)) for b in range(B)]

    in_maps = []
    for i in range(NCORES):
        b, g = divmod(i, HPC)
        cs = slice(g * DC, (g + 1) * DC)
        in_maps.append({
            "xT": xTs[b],
            "wq": np.ascontiguousarray(wq_p[:, cs].astype(bf)),
            "wk": np.ascontiguousarray(wk_p[:, cs].astype(bf)),
            "wv": np.ascontiguousarray(wv[:, cs].astype(bf)),
            "wo": np.ascontiguousarray(wo[cs, :].astype(bf)),
            "cosd": cos_dup,
            "sind": sin_signed,
            "tri": tri,
            "sel": sel,
            "vone": np.ones((128, (S // KC) * HPC), dtype=bf),
        })

    if "nc" not in _CACHE:
        _CACHE["nc"] = _build_program()
    nc = _CACHE["nc"]

    res = bass_utils.run_bass_kernel_spmd(nc, in_maps, core_ids=list(range(NCORES)))
    _CACHE["last_exec_time_ns"] = res.exec_time_ns
    _CACHE["last_res"] = res

    out = np.empty((B, S, D), dtype=np.float32)
    for b in range(B):
        acc = res.results[b * HPC]["outT"].astype(np.float32)
        for g in range(1, HPC):
            acc += res.results[b * HPC + g]["outT"].astype(np.float32)
        out[b] = acc.T
    return out


# revision 65
# speedup vs baseline: 1.3212x; 1.0750x over previous
"""Trainium2 Bass kernel for causal multi-head attention with interleaved RoPE.

Problem: B=2, S=2048, D=1024, 16 heads x 64 dims, causal, rope theta=1e4.

Sharding (8 cores): 2-way batch x 4-way head tensor-parallel.
  core i: batch b = i // 4, head group g = i % 4 (heads 4g..4g+3, dims 256).
  Each core computes q/k/v for its heads from x[b], runs causal flash
  attention, and produces a partial output projection outT [D, S].  Host
  sums the 4 partials per batch and transposes.

Performance design (v2):
  - Inputs in bf16 (halves HBM traffic); x streamed in 512-column chunks so
    projections start ~5us in instead of waiting 46us for the full load.
  - Scores: the two heads of a 128-partition group are computed as a
    row-tiled matmul pair (tile_position (0,0)/(64,0)) so both K=64
    contractions run concurrently in the PE array.
  - Both heads' score chunks live in one [128, 2, 512] PSUM tile (2 banks)
    and are EXPed by a single scalar activation -> fewer scalar instructions
    (scalar engine is the bottleneck of the attention phase; concurrent
    scalar activity also throttles the PE to ~1.2GHz, so PE work per chunk
    is halved via pairing).
  - AV: col-tiled pair (tile_position (0,0)/(0,64)) into two separate PSUM
    banks (separate banks because a matmul with start=True clears the
    has_written bits of its whole bank).
  - Softmax denominators: probs are accumulated on the Vector engine into
    sacc, reduced with M=1 ones-matmuls, inverted with one
    reciprocal_approx_fast, and broadcast back with a selection matmul --
    no DRAM round trip.
  - Output projection per q-tile right after normalize, overlapping DMA out.
"""

import os
import sys

sys.path.insert(0, "/opt/trn_rl_repo")

import numpy as np

B = 2
S = 2048
D = 1024
NH = 16
HD = 64
THETA = 10000.0
NCORES = 8
HPC = 4  # heads per core
DC = HPC * HD  # 256 dims per core
GQ = 2  # 128-partition head groups per core
QT = 512  # query tile (free dim)
NQT = S // QT
KC = 128  # key chunk (partition dim)
MASKVAL = -60.0

_CACHE = {}


def _install_axon_ntff_hook():
    """Register antenv.axon_hooks so trace=True (BASS_TRACE=1) works."""
    import types

    if "antenv.axon_hooks" in sys.modules:
        return
    m = types.ModuleType("antenv.axon_hooks")
    _hook = [None]
    m.set_axon_ntff_profile_hook = lambda h: _hook.__setitem__(0, h)
    m.get_axon_ntff_profile_hook = lambda: _hook[0]
    sys.modules["antenv.axon_hooks"] = m
    try:
        import antenv

        antenv.axon_hooks = m
        from trn_agent_boot.trn_boot import _ntff_profile_via_ctypes

        hook = _ntff_profile_via_ctypes("/opt/axon/libaxon_pjrt.so")
        if hook is not None:
            m.set_axon_ntff_profile_hook(hook)
    except Exception:
        pass


def _rope_perm_local():
    """Permutation of one head's 64 dims: original interleaved pair (2i, 2i+1)
    -> t0 at quadrant*32 + (i%16), t1 at quadrant*32 + 16 + (i%16), with
    quadrant = i // 16.  Returns perm such that new[j] = old[perm[j]]."""
    perm = np.zeros(HD, dtype=np.int64)
    for i in range(HD // 2):
        qd, r = divmod(i, 16)
        perm[qd * 32 + r] = 2 * i
        perm[qd * 32 + 16 + r] = 2 * i + 1
    return perm


def _rope_tables():
    """cos_dup/sin_signed [128, S]: per-partition rope tables matching the
    de-interleaved layout (pattern repeats every 64 partitions)."""
    inv_freq = 1.0 / (THETA ** (np.arange(0, HD, 2, dtype=np.float64) / HD))  # [32]
    pos = np.arange(S, dtype=np.float64)
    ang = pos[None, :] * inv_freq[:, None]  # [32, S]
    cos = np.cos(ang)
    sin = np.sin(ang)
    cos_dup = np.zeros((128, S), dtype=np.float32)
    sin_signed = np.zeros((128, S), dtype=np.float32)
    for p in range(128):
        d = p % HD
        qd, r0 = divmod(d, 32)
        if r0 < 16:
            i = qd * 16 + r0
            cos_dup[p] = cos[i]
            sin_signed[p] = -sin[i]
        else:
            i = qd * 16 + (r0 - 16)
            cos_dup[p] = cos[i]
            sin_signed[p] = sin[i]
    return cos_dup, sin_signed


def _build_program():
    import concourse.bass as bass
    from concourse import bacc, mybir
    import concourse.tile as tile

    f32 = mybir.dt.float32
    f32r = mybir.dt.float32r
    bf16 = mybir.dt.bfloat16
    ADD = mybir.AluOpType.add
    MULT = mybir.AluOpType.mult
    EXP = mybir.ActivationFunctionType.Exp
    SWAP16 = [(j + 16) % 32 for j in range(32)]
    DK = D // 128  # 8 contraction chunks

    nc = bacc.Bacc("TRN2", target_bir_lowering=False, debug=False)
    xT = nc.dram_tensor("xT", [D, S], bf16, kind="ExternalInput").ap()
    wq = nc.dram_tensor("wq", [D, DC], bf16, kind="ExternalInput").ap()
    wk = nc.dram_tensor("wk", [D, DC], bf16, kind="ExternalInput").ap()
    wv = nc.dram_tensor("wv", [D, DC], bf16, kind="ExternalInput").ap()
    wo = nc.dram_tensor("wo", [DC, D], bf16, kind="ExternalInput").ap()
    cosd = nc.dram_tensor("cosd", [128, S], bf16, kind="ExternalInput").ap()
    sind = nc.dram_tensor("sind", [128, S], bf16, kind="ExternalInput").ap()
    tri = nc.dram_tensor("tri", [KC, KC], bf16, kind="ExternalInput").ap()
    sel = nc.dram_tensor("sel", [128, GQ * 128], bf16,
                         kind="ExternalInput").ap()
    vone = nc.dram_tensor("vone", [128, (S // KC) * HPC], bf16,
                          kind="ExternalInput").ap()
    outT = nc.dram_tensor("outT", [D, S], bf16, kind="ExternalOutput").ap()

    with tile.TileContext(nc) as tc:
        with tc.tile_pool(name="const", bufs=1) as const:
            cos_sb = const.tile([128, S], bf16)
            sin_sb = const.tile([128, S], bf16)
            tri_sb = const.tile([KC, KC], bf16)
            wq_sb = const.tile([128, DK, DC], bf16)
            wk_sb = const.tile([128, DK, DC], bf16)
            wv_sb = const.tile([128, DK, DC], bf16)
            wo_sb = const.tile([128, GQ, D], bf16)
            xT_sb = const.tile([128, DK, S], bf16)
            qT_sb = const.tile([128, GQ, S], bf16)
            kT_sb = const.tile([128, GQ, S], bf16)
            vaug_sb = const.tile([128, S // KC, HPC * (HD + 1)], bf16)
            oT_sb = const.tile([128, GQ, S], bf16)
            sums_sb = const.tile([128, S], f32)
            recip_sb = const.tile([128, S], f32)
            recip_bf = const.tile([128, S], bf16)
            sel_sb = const.tile([128, GQ, 128], bf16)

            # DMA order tuned so phase 1 can start ~5us in.
            xTr = xT.rearrange("(o p) n -> p o n", p=128)
            nc.sync.dma_start(wq_sb, wq.rearrange("(o p) n -> p o n", p=128))
            nc.sync.dma_start(xT_sb[:, :, 0:QT], xTr[:, :, 0:QT])
            nc.sync.dma_start(wk_sb, wk.rearrange("(o p) n -> p o n", p=128))
            nc.sync.dma_start(cos_sb[:, 0:QT], cosd[:, 0:QT])
            nc.sync.dma_start(sin_sb[:, 0:QT], sind[:, 0:QT])
            nc.sync.dma_start(wv_sb, wv.rearrange("(o p) n -> p o n", p=128))
            nc.sync.dma_start(tri_sb, tri)
            nc.sync.dma_start(
                vaug_sb[:, :, HD::(HD + 1)],
                vone.rearrange("p (a b) -> p a b", a=S // KC))
            nc.sync.dma_start(
                sel_sb, sel.rearrange("p (c n) -> p c n", c=GQ))
            for qt in range(1, NQT):
                q0 = qt * QT
                nc.sync.dma_start(xT_sb[:, :, q0:q0 + QT], xTr[:, :, q0:q0 + QT])
                nc.sync.dma_start(cos_sb[:, q0:q0 + QT], cosd[:, q0:q0 + QT])
                nc.sync.dma_start(sin_sb[:, q0:q0 + QT], sind[:, q0:q0 + QT])
            nc.sync.dma_start(wo_sb, wo.rearrange("(o p) n -> p o n", p=128))

            # ---- Phase 1: q/k/v projections + rope (PE warm, scalar idle) ----
            with tc.tile_pool(name="p1", bufs=2, space="PSUM") as p1, \
                 tc.tile_pool(name="tmp1", bufs=3) as tmp1:
                def rope(ps, dst, q0):
                    # pure-bf16 chain after one psum cast: 2x DVE throughput
                    qb = tmp1.tile([128, QT], bf16, tag="qb")
                    nc.vector.tensor_copy(out=qb, in_=ps)
                    shuf = tmp1.tile([128, QT], bf16, tag="shuf")
                    nc.vector.stream_shuffle(shuf, qb, SWAP16)
                    m1 = tmp1.tile([128, QT], bf16, tag="m1")
                    nc.vector.tensor_tensor(m1, qb, cos_sb[:, q0:q0 + QT], MULT)
                    m2 = tmp1.tile([128, QT], bf16, tag="m2")
                    nc.vector.tensor_tensor(m2, shuf, sin_sb[:, q0:q0 + QT], MULT)
                    nc.vector.tensor_tensor(dst, m1, m2, ADD)

                import concourse.bass as _b
                for qt in range(NQT):
                    q0 = qt * QT
                    for g in range(GQ):
                        ps_q = p1.tile([128, QT], f32, tag="q")
                        for kc in range(DK):
                            nc.tensor.matmul(
                                ps_q, wq_sb[:, kc, g * 128:(g + 1) * 128],
                                xT_sb[:, kc, q0:q0 + QT],
                                start=(kc == 0), stop=(kc == DK - 1))
                        ps_k = p1.tile([128, QT], f32, tag="k")
                        for kc in range(DK):
                            nc.tensor.matmul(
                                ps_k, wk_sb[:, kc, g * 128:(g + 1) * 128],
                                xT_sb[:, kc, q0:q0 + QT],
                                start=(kc == 0), stop=(kc == DK - 1))
                        rope(ps_q, qT_sb[:, g, q0:q0 + QT], q0)
                        rope(ps_k, kT_sb[:, g, q0:q0 + QT], q0)
                    for rc in range(QT // KC):
                        r0 = q0 + rc * KC
                        ps_v = p1.tile([128, DC], f32, tag="v", bufs=4)
                        for kc in range(DK):
                            nc.tensor.matmul(
                                ps_v, xT_sb[:, kc, r0:r0 + KC],
                                wv_sb[:, kc, :],
                                start=(kc == 0), stop=(kc == DK - 1))
                        # one strided copy: psum [128,(h d)] -> vaug 65-pitch
                        vdst = vaug_sb[:, r0 // KC, 0:HD]
                        dst3 = _b.AP(tensor=vdst.tensor, offset=vdst.offset,
                                     ap=[list(vdst.ap[0]), [HD + 1, HPC],
                                         [1, HD]])
                        src3 = _b.AP(tensor=ps_v.tensor, offset=ps_v.offset,
                                     ap=[list(ps_v.ap[0]), [HD, HPC],
                                         [1, HD]])
                        nc.vector.tensor_copy(out=dst3, in_=src3)

            # ---- Phase 3: causal flash attention (S^T orientation) ----
            # Scores for the two heads of a group are a row-tiled concurrent
            # pair into one 2-bank PSUM tile, EXPed by a single scalar op.
            # AV uses the ones-row trick (M=65): psum row 64 = softmax sums.
            with tc.tile_pool(name="pss", bufs=2, space="PSUM") as pss, \
                 tc.tile_pool(name="po", bufs=2, space="PSUM") as po, \
                 tc.tile_pool(name="ppr", bufs=5) as ppr:
                # Software-pipelined: AV of chunk i is emitted after the
                # S-pair of chunk i+2, so the PE never queues behind EXP(i)
                # even when the diag-chunk tri-add delays EXP.
                chunks = []
                for g in range(GQ):
                    for qt in range(NQT):
                        nkc = (qt * QT + QT) // KC
                        for kc in range(nkc):
                            chunks.append((g, qt, kc, nkc))

                # sums_sb init to 1.0 (memset >1 column miscompiles): garbage
                # lanes must stay finite-nonzero for the batched reciprocal.
                # Done here: phase-1's vector engine is saturated, attention's
                # has slack, and the first sums-row write lands much later.
                for qt in range(NQT):
                    q0 = qt * QT
                    nc.vector.tensor_scalar(
                        sums_sb[:, q0:q0 + QT], cos_sb[:, q0:q0 + QT],
                        0.0, 1.0, MULT, ADD)

                state = {}  # (g, qt) -> ps_o pair
                pendq = []  # [(g, qt, kc, nkc, probs, qlo)]

                def emit_av(p):
                    g, qt, kc, nkc, probs, qlo = p
                    q0 = qt * QT
                    for a in range(2):
                        h = 2 * g + a
                        nc.tensor.matmul(
                            state[(g, qt)][a][:, qlo:QT],
                            vaug_sb[:, kc, h * (HD + 1):(h + 1) * (HD + 1)],
                            probs[:, a, qlo:QT],
                            start=(kc == 0), stop=(kc == nkc - 1))
                    if kc == nkc - 1:
                        for a in range(2):
                            h = 2 * g + a
                            nc.vector.tensor_copy(
                                out=oT_sb[a * HD:(a + 1) * HD, g, q0:q0 + QT],
                                in_=state[(g, qt)][a][0:HD, :])
                            nc.vector.tensor_copy(
                                out=sums_sb[32 * h:32 * h + 1, q0:q0 + QT],
                                in_=state[(g, qt)][a][HD:HD + 1, :])
                        del state[(g, qt)]
                        if (g, qt) == (GQ - 1, NQT - 2):
                            # all sums except qt=NQT-1 are final: invert the
                            # first 3/4 while the last q-tile is still running
                            nc.vector.reciprocal_approx_fast(
                                recip_sb[:, 0:(NQT - 1) * QT],
                                sums_sb[:, 0:(NQT - 1) * QT])
                            nc.vector.tensor_copy(
                                out=recip_bf[:, 0:(NQT - 1) * QT],
                                in_=recip_sb[:, 0:(NQT - 1) * QT])

                for g, qt, kc, nkc in chunks:
                    q0 = qt * QT
                    k0 = kc * KC
                    j = k0 - q0
                    qlo = max(0, j)
                    if kc == 0:
                        state[(g, qt)] = [
                            po.tile([HD + 1, QT], f32, tag=f"o{a}",
                                    name=f"ps_o{g}_{qt}_{a}")
                            for a in range(2)]
                    ps_s = pss.tile([128, 2, QT], f32, tag="s")
                    for a in range(2):
                        nc.tensor.matmul(
                            ps_s[:, a, qlo:QT],
                            kT_sb[a * HD:(a + 1) * HD, g, k0:k0 + KC],
                            qT_sb[a * HD:(a + 1) * HD, g, q0 + qlo:q0 + QT],
                            start=True, stop=True)
                    if len(pendq) >= 2:
                        emit_av(pendq.pop(0))
                    probs = ppr.tile([128, 2, QT], bf16, tag="p")
                    nc.scalar.activation(
                        probs[:, :, qlo:QT], ps_s[:, :, qlo:QT], EXP)
                    if j >= 0:
                        # mask the diag block on the probs (0/1 multiply):
                        # keeps the DVE off the S->EXP critical chain
                        for a in range(2):
                            nc.vector.tensor_tensor(
                                probs[:, a, qlo:qlo + KC],
                                probs[:, a, qlo:qlo + KC], tri_sb, MULT)
                    pendq.append((g, qt, kc, nkc, probs, qlo))
                for p in pendq:
                    emit_av(p)

            # ---- Phase 4+5: normalize and output projection ----
            with tc.tile_pool(name="p5", bufs=4, space="PSUM") as p5, \
                 tc.tile_pool(name="pr", bufs=2, space="PSUM") as pr, \
                 tc.tile_pool(name="p5s", bufs=6) as p5s:
                # per q-tile: normalize then project.  qt 0..NQT-2's recip was
                # computed inside the attention loop; qt NQT-1's comes first
                # here (it only needs the last sums rows).
                q3 = (NQT - 1) * QT
                for qt in range(NQT):
                    q0 = qt * QT
                    if qt == NQT - 1:
                        nc.vector.reciprocal_approx_fast(
                            recip_sb[:, q3:S], sums_sb[:, q3:S])
                        nc.vector.tensor_copy(
                            out=recip_bf[:, q3:S], in_=recip_sb[:, q3:S])
                    for g in range(GQ):
                        ps_r = pr.tile([128, QT], f32, tag="r")
                        nc.tensor.matmul(ps_r, sel_sb[:, g, :],
                                         recip_bf[:, q0:q0 + QT],
                                         start=True, stop=True)
                        nc.vector.tensor_tensor(
                            oT_sb[:, g, q0:q0 + QT], oT_sb[:, g, q0:q0 + QT],
                            ps_r, MULT)
                    for ec in range(D // 128):
                        ps = p5.tile([128, QT], f32, tag="f")
                        for g in range(GQ):
                            nc.tensor.matmul(
                                ps, wo_sb[:, g, ec * 128:(ec + 1) * 128],
                                oT_sb[:, g, q0:q0 + QT],
                                start=(g == 0), stop=(g == GQ - 1))
                        ob = p5s.tile([128, QT], bf16, tag="ob")
                        if ec % 2 == 0:
                            nc.scalar.copy(out=ob, in_=ps)
                        else:
                            nc.vector.tensor_copy(out=ob, in_=ps)
                        nc.sync.dma_start(
                            outT[ec * 128:(ec + 1) * 128, q0:q0 + QT], ob)

    nc.finalize()
    return nc


def kernel(x, wq, wk, wv, wo):
    import ml_dtypes
    from concourse import bass_utils

    if os.environ.get("BASS_TRACE"):
        _install_axon_ntff_hook()

    bf = ml_dtypes.bfloat16
    x = np.asarray(x, dtype=np.float32)
    wq = np.asarray(wq, dtype=np.float32)
    wk = np.asarray(wk, dtype=np.float32)
    wv = np.asarray(wv, dtype=np.float32)
    wo = np.asarray(wo, dtype=np.float32)

    # Host prep: weight slicing + rope column permutation + tables.
    perm_l = _rope_perm_local()
    perm = np.concatenate([h * HD + perm_l for h in range(NH)])  # [D]
    scale = 1.0 / np.sqrt(HD)
    wq_p = np.ascontiguousarray(wq[:, perm] * scale)
    wk_p = np.ascontiguousarray(wk[:, perm])
    cos_dup, sin_signed = _rope_tables()
    cos_dup = cos_dup.astype(bf)
    sin_signed = sin_signed.astype(bf)
    kl = np.arange(KC)[:, None]
    ql = np.arange(KC)[None, :]
    tri = np.where(ql >= kl, 1.0, 0.0).astype(bf)  # 0/1 probs mask

    # sel[p_src, g*128 + p_dst] = 1 iff p_src == 32 * (2g + p_dst//64):
    # broadcast head (2g + p_dst//64)'s recip row onto all its 64 dims.
    sel = np.zeros((128, GQ, 128), dtype=np.float32)
    for g in range(GQ):
        for a in range(2):
            sel[32 * (2 * g + a), g, a * HD:(a + 1) * HD] = 1.0
    sel = np.ascontiguousarray(sel.reshape(128, GQ * 128).astype(bf))

    xTs = [np.ascontiguousarray(x[b].T.astype(bf)) for b in range(B)]

    in_maps = []
    for i in range(NCORES):
        b, g = divmod(i, HPC)
        cs = slice(g * DC, (g + 1) * DC)
        in_maps.append({
            "xT": xTs[b],
            "wq": np.ascontiguousarray(wq_p[:, cs].astype(bf)),
            "wk": np.ascontiguousarray(wk_p[:, cs].astype(bf)),
            "wv": np.ascontiguousarray(wv[:, cs].astype(bf)),
            "wo": np.ascontiguousarray(wo[cs, :].astype(bf)),
            "cosd": cos_dup,
            "sind": sin_signed,
            "tri": tri,
            "sel": sel,
            "vone": np.ones((128, (S // KC) * HPC), dtype=bf),
        })

    if "nc" not in _CACHE:
        _CACHE["nc"] = _build_program()
    nc = _CACHE["nc"]

    res = bass_utils.run_bass_kernel_spmd(nc, in_maps, core_ids=list(range(NCORES)))
    _CACHE["last_exec_time_ns"] = res.exec_time_ns
    _CACHE["last_res"] = res

    out = np.empty((B, S, D), dtype=np.float32)
    for b in range(B):
        acc = res.results[b * HPC]["outT"].astype(np.float32)
        for g in range(1, HPC):
            acc += res.results[b * HPC + g]["outT"].astype(np.float32)
        out[b] = acc.T
    return out
